# revision 20
# baseline (speedup 1.0000x reference)
"""HyperGNN message-passing kernel v4 (nn_Conv_13778255086166) for 8 TRN2 cores.

Reference computation:
    Xp    = X @ W                                   [N, 64]
    Xe_s  = segment_sum(Xp[vertex], edges, E);  cnt = segment_sum(1, edges, E)
    Ze    = (homo / max(cnt,1)) * Xe_s              [E, 64]
    att_s = segment_sum(homo[edges], vertex, N)
    Xv    = segment_sum(Ze[edges], vertex, N) / att_s
    out   = row_l2_normalize(Xp + Xv)

Distribution: incidence lists sharded by vertex range (core k owns nodes
[k*12500, (k+1)*12500)); per-core edge partials AllReduced (bf16).

v4 vs v3:
  - variable per-tile slot caps (exact counts rounded to 128) instead of one
    global max cap: ~15% fewer gathered rows / masks / matmuls
  - one-hot masks built with pair-duplicated offsets so every DVE operand has
    a packed last dim -> 2x DVE mode (broadcast last-dim stride-0 disables it)
  - offsets for all tiles preloaded in one DMA (partition-major host layout)
  - staged SBUF accumulators with one strided DMA per phase; batched finalize
"""

from dataclasses import dataclass
from itertools import product

import numpy as np

import concourse.bacc as bacc
import concourse.mybir as mybir
import concourse.tile as tile
from concourse import bass_utils

F32 = mybir.dt.float32
BF16 = mybir.dt.bfloat16
I16 = mybir.dt.int16


@dataclass(frozen=True)
class Cfg:
    n_cores: int = 8
    N: int = 100000
    E: int = 25000
    caps1: tuple = ()   # per-edge-tile slot caps (multiples of 128)
    caps2: tuple = ()   # per-node-tile slot caps

    @staticmethod
    def from_inputs(vertex, edges, n_cores=8, N=100000, E=25000):
        """Exact per-tile slot caps (max over cores, rounded to 128)."""
        vertex = np.asarray(vertex).astype(np.int64)
        edges = np.asarray(edges).astype(np.int64)
        npc = N // n_cores
        etiles = ((E + 1 + 127) // 128 * 128) // 128
        ntiles = ((npc + 1 + 127) // 128 * 128) // 128
        m1 = np.zeros(etiles, np.int64)
        m2 = np.zeros(ntiles, np.int64)
        for k in range(n_cores):
            sel = (vertex >= k * npc) & (vertex < (k + 1) * npc)
            v_l, e_l = vertex[sel] - k * npc, edges[sel]
            m1 = np.maximum(m1, np.bincount(e_l >> 7, minlength=etiles))
            m2 = np.maximum(m2, np.bincount(v_l >> 7, minlength=ntiles))
        r = lambda x: int(max(128, (x + 127) // 128 * 128))
        return Cfg(n_cores=n_cores, N=N, E=E,
                   caps1=tuple(r(x) for x in m1),
                   caps2=tuple(r(x) for x in m2))

    @property
    def npc(self):
        assert self.N % self.n_cores == 0
        return self.N // self.n_cores

    @property
    def npcp(self):  # padded, with at least one spare zero row
        return (self.npc + 1 + 127) // 128 * 128

    @property
    def ntiles(self):
        return self.npcp // 128

    @property
    def ep(self):
        return (self.E + 1 + 127) // 128 * 128

    @property
    def etiles(self):
        return self.ep // 128


def _bf16():
    import ml_dtypes
    return ml_dtypes.bfloat16


def wrap_idx(idx: np.ndarray) -> np.ndarray:
    """int16 index layout for dma_gather: element j at [j%16, j//16],
    replicated across the 8 16-partition groups (one per Q7 cpu)."""
    s = idx.shape[0]
    assert s % 16 == 0
    w = np.ascontiguousarray(idx.astype(np.int16).reshape(-1, 16).T)
    return np.tile(w, (8, 1))


def prep_core_inputs(cfg: Cfg, k: int, X, W, homo, vertex, edges):
    """Host-side shard/sort/pad for core k (index/layout reorganization only)."""
    bf16 = _bf16()
    npc, npcp = cfg.npc, cfg.npcp
    vertex = np.asarray(vertex)
    edges = np.asarray(edges)
    sel = (vertex >= k * npc) & (vertex < (k + 1) * npc)
    v_l = (vertex[sel] - k * npc).astype(np.int64)
    e_l = edges[sel].astype(np.int64)

    def build(seg, other, tiles_n, caps, pad_gather):
        caps = np.asarray(caps)
        o = np.argsort(seg, kind="stable")
        s, g = seg[o], other[o]
        t_of = s >> 7
        counts = np.bincount(t_of, minlength=tiles_n)
        assert (counts <= caps).all(), (counts.max(), caps.max())
        starts = np.cumsum(counts) - counts
        capoff = np.cumsum(caps) - caps
        rank = np.arange(len(s)) - starts[t_of]
        dest = capoff[t_of] + rank
        S = int(caps.sum())
        gi = np.full(S, pad_gather, np.int64)
        off = np.zeros(S, np.float32)
        gi[dest] = g
        off[dest] = (s & 127).astype(np.float32)
        # flat idx blocks (tile-contiguous), partition-major paired offsets
        idx_flat = np.concatenate(
            [wrap_idx(gi[capoff[t]:capoff[t] + caps[t]]).ravel()
             for t in range(tiles_n)])
        offp = np.concatenate(
            [off[capoff[t]:capoff[t] + caps[t]].reshape(caps[t] // 128, 128).T
             for t in range(tiles_n)], axis=1)          # [128, sum_c]
        off_pm = np.repeat(offp, 2, axis=1).astype(bf16)  # [128, sum_c*2]
        return idx_flat, np.ascontiguousarray(off_pm)

    # P1: segment by edge, gather by local vertex; pads gather zero row npc.
    g1, off1 = build(e_l, v_l, cfg.etiles, cfg.caps1, pad_gather=npc)
    # P2: segment by local vertex, gather by edge; pads gather zero row E.
    g2, off2 = build(v_l, e_l, cfg.ntiles, cfg.caps2, pad_gather=cfg.E)

    Xt = np.zeros((64, npcp), np.float32)
    Xt[:, :npc] = np.asarray(X)[k * npc:(k + 1) * npc].T

    homo_pad = np.zeros(cfg.ep, np.float32)
    homo_pad[:cfg.E] = np.asarray(homo)
    homo_t = np.ascontiguousarray(homo_pad.reshape(cfg.etiles, 128).T)

    # global per-edge incidence counts: pure index data -> host computes
    cnt = np.bincount(edges.astype(np.int64), minlength=cfg.E).astype(np.float32)
    cntr_pad = np.zeros(cfg.ep, np.float32)
    cntr_pad[:cfg.E] = 1.0 / np.maximum(cnt, 1.0)
    cntr_t = np.ascontiguousarray(cntr_pad.reshape(cfg.etiles, 128).T)

    iota = np.broadcast_to(np.arange(128, dtype=np.float32),
                           (128, 128)).astype(bf16).copy()

    return {
        "Xt": Xt,
        "W": np.asarray(W, dtype=np.float32),
        "homo_t": homo_t,
        "cntr_t": cntr_t,
        "iota": iota,
        "g1": g1,
        "off1": off1,
        "g2": g2,
        "off2": off2,
    }


def build_nc(cfg: Cfg, for_sim: bool = False, variant: str = "full",
             repeat: int = 1, gsplit: int = 2):
    """variant: full | nocc | p0 | p1 | p2 | p1n | p2n | nog"""
    no_g1 = variant in ("p1n", "nog")
    no_g2 = variant in ("p2n", "nog")
    variant = {"p1n": "p1", "p2n": "p2", "nog": "nocc"}.get(variant, variant)
    caps1, caps2 = cfg.caps1, cfg.caps2
    cs1 = [c // 128 for c in caps1]
    cs2 = [c // 128 for c in caps2]
    c1m, c2m = max(cs1), max(cs2)
    co1 = np.cumsum([0] + cs1)   # per-tile column offsets (128-slot units)
    co2 = np.cumsum([0] + cs2)
    sum1, sum2 = int(co1[-1]), int(co2[-1])
    nt, et = cfg.ntiles, cfg.etiles
    nc = bacc.Bacc("TRN2", target_bir_lowering=False, debug=False,
                   num_devices=1 if for_sim else cfg.n_cores,
                   num_swdge_queues=4, dynamic_dma_scratch_size=32768)

    xt_d = nc.dram_tensor("Xt", [64, cfg.npcp], F32, kind="ExternalInput")
    w_d = nc.dram_tensor("W", [64, 64], F32, kind="ExternalInput")
    homo_d = nc.dram_tensor("homo_t", [128, et], F32, kind="ExternalInput")
    cntr_d = nc.dram_tensor("cntr_t", [128, et], F32, kind="ExternalInput")
    iota_d = nc.dram_tensor("iota", [128, 128], BF16, kind="ExternalInput")
    g1_d = nc.dram_tensor("g1", [sum1 * 1024], I16, kind="ExternalInput")
    off1_d = nc.dram_tensor("off1", [128, sum1 * 2], BF16, kind="ExternalInput")
    g2_d = nc.dram_tensor("g2", [sum2 * 1024], I16, kind="ExternalInput")
    off2_d = nc.dram_tensor("off2", [128, sum2 * 2], BF16, kind="ExternalInput")
    out_d = nc.dram_tensor("out", [cfg.npcp, 64], F32, kind="ExternalOutput")

    xp_d = nc.dram_tensor("XpD", [cfg.npcp, 128], BF16, kind="Internal")
    eacc_d = nc.dram_tensor("EaccD", [cfg.ep, 64], BF16, kind="Internal")
    ered_d = nc.dram_tensor("EredD", [cfg.ep, 64], BF16, kind="Internal",
                            addr_space="Shared")
    zef_d = nc.dram_tensor("ZeFD", [cfg.ep, 128], BF16, kind="Internal")

    with tile.TileContext(nc) as tc:
        xtiles = next(d for d in range(min(7, nt), 0, -1) if nt % d == 0)
        xchunk = xtiles * 128  # phase-0 X streamed in nt/xtiles chunks
        zchunk = next(d for d in range(min(7, et), 0, -1) if et % d == 0)
        fchunk = next(d for d in range(min(14, nt), 0, -1) if nt % d == 0)
        with (
            tc.tile_pool(name="const", bufs=1) as pc,
            tc.tile_pool(name="xin", bufs=2) as px,
            tc.tile_pool(name="idx", bufs=8) as pidx,
            tc.tile_pool(name="gather", bufs=4) as pg,
            tc.tile_pool(name="onehot", bufs=3) as pm,
            tc.tile_pool(name="ze", bufs=2) as pz,
            tc.tile_pool(name="fin", bufs=2) as pf,
            tc.tile_pool(name="psum", bufs=4, space="PSUM") as pp,
        ):
            w_sb = pc.tile([64, 64], F32)
            nc.sync.dma_start(out=w_sb[:], in_=w_d[:])
            iota_sb = pc.tile([128, 64, 2], BF16)
            nc.sync.dma_start(
                out=iota_sb[:],
                in_=iota_d[:].rearrange("p (a b) -> p a b", b=2))
            homo_sb = pc.tile([128, et], F32)
            nc.sync.dma_start(out=homo_sb[:], in_=homo_d[:])
            cntr_sb = pc.tile([128, et], F32)
            nc.sync.dma_start(out=cntr_sb[:], in_=cntr_d[:])
            off1_sb = pc.tile([128, sum1, 2], BF16)
            nc.scalar.dma_start(
                out=off1_sb[:],
                in_=off1_d[:].rearrange("p (a b) -> p a b", b=2))
            off2_sb = pc.tile([128, sum2, 2], BF16)
            nc.scalar.dma_start(
                out=off2_sb[:],
                in_=off2_d[:].rearrange("p (a b) -> p a b", b=2))

            # persistent SBUF stages
            xp_all = pc.tile([128, nt, 64], F32)    # f32 Xp for phase-2 add
            acc_all = pc.tile([128, et, 64], BF16)  # phase-1 edge partials
            o_all = pc.tile([128, nt, 64], F32)     # phase-2 raw node sums
            att_all = pc.tile([128, nt], F32)       # phase-2 att sums

            # phase 0: Xp = X_local @ W -> xp_all (SBUF) and xp_d (DRAM, bf16)
            xp_view = xp_d.ap().rearrange("(t p) f -> p t f", p=128)
            for rep, cs in product(range(repeat), range(0, cfg.npcp, xchunk)):
                xc = px.tile([64, xchunk], F32, tag="xc")
                nc.sync.dma_start(out=xc[:], in_=xt_d[:, cs:cs + xchunk])
                xq = px.tile([128, xtiles, 64], BF16, tag="xq")
                for u in range(xtiles):
                    t = cs // 128 + u
                    ps = pp.tile([128, 65], F32, tag="ps")
                    nc.tensor.matmul(ps[:, 0:64], lhsT=xc[:, u * 128:(u + 1) * 128],
                                     rhs=w_sb[:], start=True, stop=True)
                    nc.vector.tensor_copy(out=xp_all[:, t, :], in_=ps[:, 0:64])
                    nc.scalar.copy(out=xq[:, u, :], in_=ps[:, 0:64])
                nc.sync.dma_start(
                    out=xp_view[:, cs // 128:cs // 128 + xtiles, 0:64], in_=xq[:])

            # phase 1: edge-tile accumulation of bf16 Xp rows.
            # Pipelined: tile s+LA's gather+mask are emitted before tile s's
            # psum copy so the in-order DVE/Q7 queues stay ahead of the PE.
            LA = 2
            p1_reps = 0 if variant in ("p0", "p2") else repeat

            def stage_f(s, cs, co, src_d, idx_d, off_sb, gtag, ctag, no_g):
                c = cs[s]
                o = int(co[s])
                gi = pidx.tile([128, 8 * c1m if gtag == "g1" else 8 * c2m],
                               I16, tag="i" + gtag)
                nc.sync.dma_start(
                    out=gi[:, 0:8 * c],
                    in_=idx_d[o * 1024:(o + c) * 1024]
                        .rearrange("(p c) -> p c", p=128))
                g = pg.tile([128, c1m if gtag == "g1" else c2m, 128], BF16,
                            tag=gtag, bufs=4)
                if no_g:
                    nc.vector.memset(g[:, 0:c, :], 0.0)
                else:
                    for h in range(gsplit):
                        a = (h * c) // gsplit
                        b = ((h + 1) * c) // gsplit
                        if a == b:
                            continue
                        nc.gpsimd.dma_gather(
                            g[:, a:b, :], src_d[:], gi[:, a * 8:b * 8],
                            (b - a) * 128, (b - a) * 128, 128,
                            single_packet=False,
                            queue_num=(gsplit * s + h) % 4)
                mt = pm.tile([128, c1m if gtag == "g1" else c2m, 64, 2], BF16,
                             tag=ctag, bufs=3)
                nc.vector.tensor_tensor(
                    out=mt[:, 0:c, :, :],
                    in0=iota_sb[:].unsqueeze(1).broadcast_to([128, c, 64, 2]),
                    in1=off_sb[:, o:o + c, :].unsqueeze(2)
                        .broadcast_to([128, c, 64, 2]),
                    op=mybir.AluOpType.is_equal)
                return g, mt

            def stage1(s):
                return stage_f(s, cs1, co1, xp_d, g1_d, off1_sb, "g1", "mt1",
                               no_g1)

            for rep in range(p1_reps):
                stage = {s: stage1(s) for s in range(min(LA, et))}
                for s in range(et):
                    if s + LA < et:
                        stage[s + LA] = stage1(s + LA)
                    g, mt = stage.pop(s)
                    ps = pp.tile([128, 65], F32, tag="ps")
                    for j in range(cs1[s]):
                        nc.tensor.matmul(
                            ps[:, 0:64],
                            lhsT=mt[:, j, :, :].rearrange("p a b -> p (a b)"),
                            rhs=g[:, j, 0:64],
                            start=(j == 0), stop=(j == cs1[s] - 1))
                    nc.vector.tensor_copy(out=acc_all[:, s, :], in_=ps[:, 0:64])
            eacc_view = eacc_d.ap().rearrange("(t p) f -> p t f", p=128)
            for rep in range(p1_reps):
                nc.sync.dma_start(out=eacc_view[:], in_=acc_all[:])

            # AllReduce edge partials (bf16)
            cc_reps = 0 if variant in ("p0", "p1") else repeat
            for rep in range(cc_reps):
                if for_sim or variant in ("nocc", "p2"):
                    nc.sync.dma_start(out=ered_d[:], in_=eacc_d[:])
                else:
                    nc.gpsimd.collective_compute(
                        "AllReduce", mybir.AluOpType.add,
                        replica_groups=[list(range(cfg.n_cores))],
                        ins=[eacc_d.ap()], outs=[ered_d.ap()],
                    )

            # Ze build: zef rows = [Ye*homo | homo | junk], chunk-batched
            scale_sb = pc.tile([128, et], F32)
            nc.vector.tensor_tensor(out=scale_sb[:], in0=homo_sb[:],
                                    in1=cntr_sb[:], op=mybir.AluOpType.mult)
            er_view = ered_d.ap().rearrange("(t p) f -> p t f", p=128)
            zf_view = zef_d.ap().rearrange("(t p) f -> p t f", p=128)
            for rep, zs in product(range(cc_reps), range(0, et, zchunk)):
                er = pz.tile([128, zchunk, 64], BF16, tag="er")
                nc.sync.dma_start(out=er[:], in_=er_view[:, zs:zs + zchunk, :])
                z = pz.tile([128, zchunk, 128], BF16, tag="z")
                nc.vector.tensor_tensor(
                    out=z[:, :, 0:64], in0=er[:],
                    in1=scale_sb[:, zs:zs + zchunk].unsqueeze(2)
                        .broadcast_to([128, zchunk, 64]),
                    op=mybir.AluOpType.mult)
                nc.vector.tensor_copy(
                    out=z[:, :, 64:65],
                    in_=homo_sb[:, zs:zs + zchunk].unsqueeze(2))
                nc.sync.dma_start(out=zf_view[:, zs:zs + zchunk, :], in_=z[:])

            # phase 2: node-tile accumulation (same pipelining as phase 1)
            def stage2(s):
                return stage_f(s, cs2, co2, zef_d, g2_d, off2_sb, "g2", "mt2",
                               no_g2)

            for rep in range(cc_reps):
                stage = {s: stage2(s) for s in range(min(LA, nt))}
                for s in range(nt):
                    if s + LA < nt:
                        stage[s + LA] = stage2(s + LA)
                    g, mt = stage.pop(s)
                    ps = pp.tile([128, 65], F32, tag="ps")
                    for j in range(cs2[s]):
                        nc.tensor.matmul(
                            ps[:],
                            lhsT=mt[:, j, :, :].rearrange("p a b -> p (a b)"),
                            rhs=g[:, j, 0:65],
                            start=(j == 0), stop=(j == cs2[s] - 1))
                    nc.vector.tensor_copy(out=o_all[:, s, :], in_=ps[:, 0:64])
                    nc.vector.tensor_copy(out=att_all[:, s:s + 1],
                                          in_=ps[:, 64:65])

            # batched finalize, fchunk node tiles at a time
            out_view = out_d.ap().rearrange("(t p) f -> p t f", p=128)
            for rep, fs in product(range(cc_reps), range(0, nt, fchunk)):
                fe = fs + fchunk
                attm = pf.tile([128, fchunk], F32, tag="attm", bufs=1)
                nc.vector.tensor_scalar_max(out=attm[:], in0=att_all[:, fs:fe],
                                            scalar1=1e-30)
                arec = pf.tile([128, fchunk], F32, tag="arec", bufs=1)
                nc.vector.reciprocal(out=arec[:], in_=attm[:])
                o3 = pf.tile([128, fchunk, 64], F32, tag="o3", bufs=1)
                nc.vector.tensor_tensor(
                    out=o3[:], in0=o_all[:, fs:fe, :],
                    in1=arec[:].unsqueeze(2).broadcast_to([128, fchunk, 64]),
                    op=mybir.AluOpType.mult)
                nc.vector.tensor_tensor(out=o3[:], in0=o3[:],
                                        in1=xp_all[:, fs:fe, :],
                                        op=mybir.AluOpType.add)
                sq = pf.tile([128, fchunk, 64], F32, tag="sq", bufs=1)
                nc.vector.tensor_tensor(out=sq[:], in0=o3[:], in1=o3[:],
                                        op=mybir.AluOpType.mult)
                rs = pf.tile([128, fchunk, 1], F32, tag="rs", bufs=1)
                nc.vector.reduce_sum(out=rs[:], in_=sq[:],
                                     axis=mybir.AxisListType.X)
                rn = pf.tile([128, fchunk, 1], F32, tag="rn", bufs=1)
                nc.scalar.sqrt(out=rn[:], in_=rs[:])
                rnm = pf.tile([128, fchunk, 1], F32, tag="rnm", bufs=1)
                nc.vector.tensor_scalar_max(out=rnm[:], in0=rn[:], scalar1=1e-30)
                rrec = pf.tile([128, fchunk, 1], F32, tag="rrec", bufs=1)
                nc.vector.reciprocal(out=rrec[:], in_=rnm[:])
                nc.vector.tensor_tensor(
                    out=o3[:], in0=o3[:],
                    in1=rrec[:].broadcast_to([128, fchunk, 64]),
                    op=mybir.AluOpType.mult)
                nc.sync.dma_start(out=out_view[:, fs:fe, :], in_=o3[:])

    nc.compile()
    return nc


_NC_CACHE = {}


def kernel(**inputs) -> np.ndarray:
    """Full inputs in, full output out. Shards across 8 NeuronCores internally."""
    X = np.asarray(inputs["X"], dtype=np.float32)
    W = np.asarray(inputs["W"], dtype=np.float32)
    homo = np.asarray(inputs["homo"], dtype=np.float32)
    vertex = np.asarray(inputs["vertex"])
    edges = np.asarray(inputs["edges"])
    cfg = Cfg.from_inputs(vertex, edges)
    assert X.shape == (cfg.N, 64) and homo.shape == (cfg.E,)

    key = cfg
    if key not in _NC_CACHE:
        _NC_CACHE[key] = build_nc(cfg)
    nc = _NC_CACHE[key]

    in_maps = [prep_core_inputs(cfg, k, X, W, homo, vertex, edges)
               for k in range(cfg.n_cores)]
    res = bass_utils.run_bass_kernel_spmd(
        nc, in_maps, core_ids=list(range(cfg.n_cores)))
    out = np.concatenate(
        [res.results[k]["out"][:cfg.npc] for k in range(cfg.n_cores)], axis=0)
    return out.astype(np.float32)


# revision 26
# speedup vs baseline: 1.0399x; 1.0399x over previous
"""HyperGNN message-passing kernel v4 (nn_Conv_13778255086166) for 8 TRN2 cores.

Reference computation:
    Xp    = X @ W                                   [N, 64]
    Xe_s  = segment_sum(Xp[vertex], edges, E);  cnt = segment_sum(1, edges, E)
    Ze    = (homo / max(cnt,1)) * Xe_s              [E, 64]
    att_s = segment_sum(homo[edges], vertex, N)
    Xv    = segment_sum(Ze[edges], vertex, N) / att_s
    out   = row_l2_normalize(Xp + Xv)

Distribution: incidence lists sharded by vertex range (core k owns nodes
[k*12500, (k+1)*12500)); per-core edge partials AllReduced (bf16).

v4 vs v3:
  - variable per-tile slot caps (exact counts rounded to 128) instead of one
    global max cap: ~15% fewer gathered rows / masks / matmuls
  - one-hot masks built with pair-duplicated offsets so every DVE operand has
    a packed last dim -> 2x DVE mode (broadcast last-dim stride-0 disables it)
  - offsets for all tiles preloaded in one DMA (partition-major host layout)
  - staged SBUF accumulators with one strided DMA per phase; batched finalize
"""

from dataclasses import dataclass
from itertools import product

import numpy as np

import concourse.bacc as bacc
import concourse.mybir as mybir
import concourse.tile as tile
from concourse import bass_utils

F32 = mybir.dt.float32
BF16 = mybir.dt.bfloat16
I16 = mybir.dt.int16


@dataclass(frozen=True)
class Cfg:
    n_cores: int = 8
    N: int = 100000
    E: int = 25000
    caps1: tuple = ()   # per-edge-tile slot caps (multiples of 128)
    caps2: tuple = ()   # per-node-tile slot caps

    @staticmethod
    def from_inputs(vertex, edges, n_cores=8, N=100000, E=25000):
        """Exact per-tile slot caps (max over cores, rounded to 128)."""
        vertex = np.asarray(vertex).astype(np.int64)
        edges = np.asarray(edges).astype(np.int64)
        npc = N // n_cores
        etiles = ((E + 1 + 127) // 128 * 128) // 128
        ntiles = ((npc + 1 + 127) // 128 * 128) // 128
        m1 = np.zeros(etiles, np.int64)
        m2 = np.zeros(ntiles, np.int64)
        for k in range(n_cores):
            sel = (vertex >= k * npc) & (vertex < (k + 1) * npc)
            v_l, e_l = vertex[sel] - k * npc, edges[sel]
            m1 = np.maximum(m1, np.bincount(e_l >> 7, minlength=etiles))
            m2 = np.maximum(m2, np.bincount(v_l >> 7, minlength=ntiles))
        r = lambda x: int(max(128, (x + 127) // 128 * 128))
        return Cfg(n_cores=n_cores, N=N, E=E,
                   caps1=tuple(r(x) for x in m1),
                   caps2=tuple(r(x) for x in m2))

    @property
    def npc(self):
        assert self.N % self.n_cores == 0
        return self.N // self.n_cores

    @property
    def npcp(self):  # padded, with at least one spare zero row
        return (self.npc + 1 + 127) // 128 * 128

    @property
    def ntiles(self):
        return self.npcp // 128

    @property
    def ep(self):
        return (self.E + 1 + 127) // 128 * 128

    @property
    def etiles(self):
        return self.ep // 128


def _bf16():
    import ml_dtypes
    return ml_dtypes.bfloat16


def wrap_idx(idx: np.ndarray) -> np.ndarray:
    """int16 index layout for dma_gather: element j at [j%16, j//16],
    replicated across the 8 16-partition groups (one per Q7 cpu)."""
    s = idx.shape[0]
    assert s % 16 == 0
    w = np.ascontiguousarray(idx.astype(np.int16).reshape(-1, 16).T)
    return np.tile(w, (8, 1))


def prep_core_inputs(cfg: Cfg, k: int, X, W, homo, vertex, edges):
    """Host-side shard/sort/pad for core k (index/layout reorganization only)."""
    bf16 = _bf16()
    npc, npcp = cfg.npc, cfg.npcp
    vertex = np.asarray(vertex)
    edges = np.asarray(edges)
    sel = (vertex >= k * npc) & (vertex < (k + 1) * npc)
    v_l = (vertex[sel] - k * npc).astype(np.int64)
    e_l = edges[sel].astype(np.int64)

    def build(seg, other, tiles_n, caps, pad_gather):
        caps = np.asarray(caps)
        o = np.argsort(seg, kind="stable")
        s, g = seg[o], other[o]
        t_of = s >> 7
        counts = np.bincount(t_of, minlength=tiles_n)
        assert (counts <= caps).all(), (counts.max(), caps.max())
        starts = np.cumsum(counts) - counts
        capoff = np.cumsum(caps) - caps
        rank = np.arange(len(s)) - starts[t_of]
        dest = capoff[t_of] + rank
        S = int(caps.sum())
        gi = np.full(S, pad_gather, np.int64)
        off = np.zeros(S, np.float32)
        gi[dest] = g
        off[dest] = (s & 127).astype(np.float32)
        # flat idx blocks (pair-of-tiles contiguous, partition-major within
        # a pair), partition-major paired offsets
        wraps = [wrap_idx(gi[capoff[t]:capoff[t] + caps[t]])
                 for t in range(tiles_n)]
        idx_flat = np.concatenate(
            [np.hstack(wraps[q:q + 2]).ravel()
             for q in range(0, tiles_n, 2)])
        offp = np.concatenate(
            [off[capoff[t]:capoff[t] + caps[t]].reshape(caps[t] // 128, 128).T
             for t in range(tiles_n)], axis=1)          # [128, sum_c]
        off_pm = np.repeat(offp, 2, axis=1).astype(bf16)  # [128, sum_c*2]
        return idx_flat, np.ascontiguousarray(off_pm)

    # P1: segment by edge, gather by local vertex; pads gather zero row npc.
    g1, off1 = build(e_l, v_l, cfg.etiles, cfg.caps1, pad_gather=npc)
    # P2: segment by local vertex, gather by edge; pads gather zero row E.
    g2, off2 = build(v_l, e_l, cfg.ntiles, cfg.caps2, pad_gather=cfg.E)

    Xt = np.zeros((64, npcp), np.float32)
    Xt[:, :npc] = np.asarray(X)[k * npc:(k + 1) * npc].T

    homo_pad = np.zeros(cfg.ep, np.float32)
    homo_pad[:cfg.E] = np.asarray(homo)
    homo_t = np.ascontiguousarray(homo_pad.reshape(cfg.etiles, 128).T)

    # global per-edge incidence counts: pure index data -> host computes
    cnt = np.bincount(edges.astype(np.int64), minlength=cfg.E).astype(np.float32)
    cntr_pad = np.zeros(cfg.ep, np.float32)
    cntr_pad[:cfg.E] = 1.0 / np.maximum(cnt, 1.0)
    cntr_t = np.ascontiguousarray(cntr_pad.reshape(cfg.etiles, 128).T)

    iota = np.broadcast_to(np.arange(128, dtype=np.float32),
                           (128, 128)).astype(bf16).copy()

    return {
        "Xt": Xt,
        "W": np.asarray(W, dtype=np.float32),
        "homo_t": homo_t,
        "cntr_t": cntr_t,
        "iota": iota,
        "g1": g1,
        "off1": off1,
        "g2": g2,
        "off2": off2,
    }


def build_nc(cfg: Cfg, for_sim: bool = False, variant: str = "full",
             repeat: int = 1, gsplit: int = 2):
    """variant: full | nocc | p0 | p1 | p2 | p1n | p2n | nog"""
    no_g1 = variant in ("p1n", "nog")
    no_g2 = variant in ("p2n", "nog")
    variant = {"p1n": "p1", "p2n": "p2", "nog": "nocc"}.get(variant, variant)
    caps1, caps2 = cfg.caps1, cfg.caps2
    cs1 = [c // 128 for c in caps1]
    cs2 = [c // 128 for c in caps2]
    c1m, c2m = max(cs1), max(cs2)
    co1 = np.cumsum([0] + cs1)   # per-tile column offsets (128-slot units)
    co2 = np.cumsum([0] + cs2)
    sum1, sum2 = int(co1[-1]), int(co2[-1])
    nt, et = cfg.ntiles, cfg.etiles
    nc = bacc.Bacc("TRN2", target_bir_lowering=False, debug=False,
                   num_devices=1 if for_sim else cfg.n_cores,
                   num_swdge_queues=4, dynamic_dma_scratch_size=32768)

    xt_d = nc.dram_tensor("Xt", [64, cfg.npcp], F32, kind="ExternalInput")
    w_d = nc.dram_tensor("W", [64, 64], F32, kind="ExternalInput")
    homo_d = nc.dram_tensor("homo_t", [128, et], F32, kind="ExternalInput")
    cntr_d = nc.dram_tensor("cntr_t", [128, et], F32, kind="ExternalInput")
    iota_d = nc.dram_tensor("iota", [128, 128], BF16, kind="ExternalInput")
    g1_d = nc.dram_tensor("g1", [sum1 * 1024], I16, kind="ExternalInput")
    off1_d = nc.dram_tensor("off1", [128, sum1 * 2], BF16, kind="ExternalInput")
    g2_d = nc.dram_tensor("g2", [sum2 * 1024], I16, kind="ExternalInput")
    off2_d = nc.dram_tensor("off2", [128, sum2 * 2], BF16, kind="ExternalInput")
    out_d = nc.dram_tensor("out", [cfg.npcp, 64], F32, kind="ExternalOutput")

    xp_d = nc.dram_tensor("XpD", [cfg.npcp, 128], BF16, kind="Internal")
    eacc_d = nc.dram_tensor("EaccD", [cfg.ep, 64], BF16, kind="Internal")
    ered_d = nc.dram_tensor("EredD", [cfg.ep, 64], BF16, kind="Internal",
                            addr_space="Shared")
    zef_d = nc.dram_tensor("ZeFD", [cfg.ep, 128], BF16, kind="Internal")

    with tile.TileContext(nc) as tc:
        xtiles = next(d for d in range(min(7, nt), 0, -1) if nt % d == 0)
        xchunk = xtiles * 128  # phase-0 X streamed in nt/xtiles chunks
        zchunk = next(d for d in range(min(7, et), 0, -1) if et % d == 0)
        fchunk = next(d for d in range(min(14, nt), 0, -1) if nt % d == 0)
        with (
            tc.tile_pool(name="const", bufs=1) as pc,
            tc.tile_pool(name="xin", bufs=2) as px,
            tc.tile_pool(name="idx", bufs=8) as pidx,
            tc.tile_pool(name="gather", bufs=4) as pg,
            tc.tile_pool(name="onehot", bufs=3) as pm,
            tc.tile_pool(name="ze", bufs=2) as pz,
            tc.tile_pool(name="fin", bufs=2) as pf,
            tc.tile_pool(name="psum", bufs=4, space="PSUM") as pp,
        ):
            w_sb = pc.tile([64, 64], F32)
            nc.sync.dma_start(out=w_sb[:], in_=w_d[:])
            iota_sb = pc.tile([128, 64, 2], BF16)
            nc.sync.dma_start(
                out=iota_sb[:],
                in_=iota_d[:].rearrange("p (a b) -> p a b", b=2))
            homo_sb = pc.tile([128, et], F32)
            nc.sync.dma_start(out=homo_sb[:], in_=homo_d[:])
            cntr_sb = pc.tile([128, et], F32)
            nc.sync.dma_start(out=cntr_sb[:], in_=cntr_d[:])
            off1_sb = pc.tile([128, sum1, 2], BF16)
            nc.scalar.dma_start(
                out=off1_sb[:],
                in_=off1_d[:].rearrange("p (a b) -> p a b", b=2))
            off2_sb = pc.tile([128, sum2, 2], BF16)
            nc.scalar.dma_start(
                out=off2_sb[:],
                in_=off2_d[:].rearrange("p (a b) -> p a b", b=2))

            # persistent SBUF stages
            xp_all = pc.tile([128, nt, 64], F32)    # f32 Xp for phase-2 add
            acc_all = pc.tile([128, et, 64], BF16)  # phase-1 edge partials
            o_all = pc.tile([128, nt, 65], F32)     # phase-2 sums (col 64=att)

            # phase 0: Xp = X_local @ W -> xp_all (SBUF) and xp_d (DRAM, bf16)
            xp_view = xp_d.ap().rearrange("(t p) f -> p t f", p=128)
            for rep, cs in product(range(repeat), range(0, cfg.npcp, xchunk)):
                xc = px.tile([64, xchunk], F32, tag="xc")
                nc.sync.dma_start(out=xc[:], in_=xt_d[:, cs:cs + xchunk])
                xq = px.tile([128, xtiles, 64], BF16, tag="xq")
                for u in range(xtiles):
                    t = cs // 128 + u
                    ps = pp.tile([128, 65], F32, tag="ps", bufs=2)
                    nc.tensor.matmul(ps[:, 0:64], lhsT=xc[:, u * 128:(u + 1) * 128],
                                     rhs=w_sb[:], start=True, stop=True)
                    nc.vector.tensor_copy(out=xp_all[:, t, :], in_=ps[:, 0:64])
                    nc.scalar.copy(out=xq[:, u, :], in_=ps[:, 0:64])
                nc.sync.dma_start(
                    out=xp_view[:, cs // 128:cs // 128 + xtiles, 0:64], in_=xq[:])

            # phases 1/2: pair-of-tiles granularity. One idx DMA, one mask
            # is_equal, ~768-idx gathers (crossing tile boundaries), one
            # psum->SBUF copy per pair. Pipelined one pair ahead so the
            # in-order DVE/Q7 queues stay ahead of the PE matmul stream.
            LA = 1
            p1_reps = 0 if variant in ("p0", "p2") else repeat
            qctr = [0]

            def stage_pair(q, cs, co, pw, src_d, idx_d, off_sb, gtag, ctag,
                           no_g):
                t0 = 2 * q
                tiles = list(range(t0, min(t0 + 2, len(cs))))
                o = int(co[t0])
                C = sum(cs[t] for t in tiles)
                gi = pidx.tile([128, 8 * pw], I16, tag="i" + gtag)
                nc.sync.dma_start(
                    out=gi[:, 0:8 * C],
                    in_=idx_d[o * 1024:(o + C) * 1024]
                        .rearrange("(p c) -> p c", p=128))
                g = pg.tile([128, pw, 128], BF16, tag=gtag, bufs=2)
                if no_g:
                    nc.vector.memset(g[:, 0:C, :], 0.0)
                else:
                    nsp = max(1, (C + 5) // 6)
                    for h in range(nsp):
                        a = (h * C) // nsp
                        b = ((h + 1) * C) // nsp
                        if a == b:
                            continue
                        nc.gpsimd.dma_gather(
                            g[:, a:b, :], src_d[:], gi[:, a * 8:b * 8],
                            (b - a) * 128, (b - a) * 128, 128,
                            single_packet=False,
                            queue_num=qctr[0] % 4)
                        qctr[0] += 1
                mt = pm.tile([128, pw, 64, 2], BF16, tag=ctag, bufs=2)
                nc.vector.tensor_tensor(
                    out=mt[:, 0:C, :, :],
                    in0=iota_sb[:].unsqueeze(1).broadcast_to([128, C, 64, 2]),
                    in1=off_sb[:, o:o + C, :].unsqueeze(2)
                        .broadcast_to([128, C, 64, 2]),
                    op=mybir.AluOpType.is_equal)
                return g, mt

            def run_phase(cs, co, pw, src_d, idx_d, off_sb, gtag, ctag, no_g,
                          fcols, sink):
                ntl = len(cs)
                nq = (ntl + 1) // 2

                def stage(q):
                    return stage_pair(q, cs, co, pw, src_d, idx_d, off_sb,
                                      gtag, ctag, no_g)

                pend = {q: stage(q) for q in range(min(LA + 1, nq))}
                for q in range(nq):
                    if q + LA + 1 < nq:
                        pend[q + LA + 1] = stage(q + LA + 1)
                    g, mt = pend.pop(q)
                    tiles = list(range(2 * q, min(2 * q + 2, ntl)))
                    ps = pp.tile([128, 2, 65], F32, tag="ps2t")
                    jj = 0
                    for i, t in enumerate(tiles):
                        for j in range(cs[t]):
                            nc.tensor.matmul(
                                ps[:, i, 0:fcols],
                                lhsT=mt[:, jj, :, :]
                                    .rearrange("p a b -> p (a b)"),
                                rhs=g[:, jj, 0:fcols],
                                start=(j == 0), stop=(j == cs[t] - 1))
                            jj += 1
                    sink(ps, tiles)

            def sink1(ps, tiles):
                nc.vector.tensor_copy(
                    out=acc_all[:, tiles[0]:tiles[0] + len(tiles), :],
                    in_=ps[:, 0:len(tiles), 0:64])

            for rep in range(p1_reps):
                run_phase(cs1, co1, 2 * c1m, xp_d, g1_d, off1_sb, "g1", "mt1",
                          no_g1, 64, sink1)
            eacc_view = eacc_d.ap().rearrange("(t p) f -> p t f", p=128)
            for rep in range(p1_reps):
                nc.sync.dma_start(out=eacc_view[:], in_=acc_all[:])

            # AllReduce edge partials (bf16)
            cc_reps = 0 if variant in ("p0", "p1") else repeat
            for rep in range(cc_reps):
                if for_sim or variant in ("nocc", "p2"):
                    nc.sync.dma_start(out=ered_d[:], in_=eacc_d[:])
                else:
                    nc.gpsimd.collective_compute(
                        "AllReduce", mybir.AluOpType.add,
                        replica_groups=[list(range(cfg.n_cores))],
                        ins=[eacc_d.ap()], outs=[ered_d.ap()],
                    )

            # Ze build: zef rows = [Ye*homo | homo | junk], chunk-batched
            scale_sb = pc.tile([128, et], F32)
            nc.vector.tensor_tensor(out=scale_sb[:], in0=homo_sb[:],
                                    in1=cntr_sb[:], op=mybir.AluOpType.mult)
            er_view = ered_d.ap().rearrange("(t p) f -> p t f", p=128)
            zf_view = zef_d.ap().rearrange("(t p) f -> p t f", p=128)
            for rep, zs in product(range(cc_reps), range(0, et, zchunk)):
                er = pz.tile([128, zchunk, 64], BF16, tag="er")
                nc.sync.dma_start(out=er[:], in_=er_view[:, zs:zs + zchunk, :])
                z = pz.tile([128, zchunk, 128], BF16, tag="z")
                nc.vector.tensor_tensor(
                    out=z[:, :, 0:64], in0=er[:],
                    in1=scale_sb[:, zs:zs + zchunk].unsqueeze(2)
                        .broadcast_to([128, zchunk, 64]),
                    op=mybir.AluOpType.mult)
                nc.vector.tensor_copy(
                    out=z[:, :, 64:65],
                    in_=homo_sb[:, zs:zs + zchunk].unsqueeze(2))
                nc.sync.dma_start(out=zf_view[:, zs:zs + zchunk, :], in_=z[:])

            # phase 2: node-tile accumulation (same pairing as phase 1)
            def sink2(ps, tiles):
                nc.vector.tensor_copy(
                    out=o_all[:, tiles[0]:tiles[0] + len(tiles), :],
                    in_=ps[:, 0:len(tiles), :])

            for rep in range(cc_reps):
                run_phase(cs2, co2, 2 * c2m, zef_d, g2_d, off2_sb, "g2", "mt2",
                          no_g2, 65, sink2)

            # batched finalize, fchunk node tiles at a time
            out_view = out_d.ap().rearrange("(t p) f -> p t f", p=128)
            for rep, fs in product(range(cc_reps), range(0, nt, fchunk)):
                fe = fs + fchunk
                attm = pf.tile([128, fchunk, 1], F32, tag="attm", bufs=1)
                nc.vector.tensor_scalar_max(out=attm[:],
                                            in0=o_all[:, fs:fe, 64:65],
                                            scalar1=1e-30)
                arec = pf.tile([128, fchunk, 1], F32, tag="arec", bufs=1)
                nc.vector.reciprocal(out=arec[:], in_=attm[:])
                o3 = pf.tile([128, fchunk, 64], F32, tag="o3", bufs=1)
                nc.vector.tensor_tensor(
                    out=o3[:], in0=o_all[:, fs:fe, 0:64],
                    in1=arec[:].broadcast_to([128, fchunk, 64]),
                    op=mybir.AluOpType.mult)
                nc.vector.tensor_tensor(out=o3[:], in0=o3[:],
                                        in1=xp_all[:, fs:fe, :],
                                        op=mybir.AluOpType.add)
                sq = pf.tile([128, fchunk, 64], F32, tag="sq", bufs=1)
                nc.vector.tensor_tensor(out=sq[:], in0=o3[:], in1=o3[:],
                                        op=mybir.AluOpType.mult)
                rs = pf.tile([128, fchunk, 1], F32, tag="rs", bufs=1)
                nc.vector.reduce_sum(out=rs[:], in_=sq[:],
                                     axis=mybir.AxisListType.X)
                rn = pf.tile([128, fchunk, 1], F32, tag="rn", bufs=1)
                nc.scalar.sqrt(out=rn[:], in_=rs[:])
                rnm = pf.tile([128, fchunk, 1], F32, tag="rnm", bufs=1)
                nc.vector.tensor_scalar_max(out=rnm[:], in0=rn[:], scalar1=1e-30)
                rrec = pf.tile([128, fchunk, 1], F32, tag="rrec", bufs=1)
                nc.vector.reciprocal(out=rrec[:], in_=rnm[:])
                nc.vector.tensor_tensor(
                    out=o3[:], in0=o3[:],
                    in1=rrec[:].broadcast_to([128, fchunk, 64]),
                    op=mybir.AluOpType.mult)
                nc.sync.dma_start(out=out_view[:, fs:fe, :], in_=o3[:])

    nc.compile()
    return nc


_NC_CACHE = {}


def kernel(**inputs) -> np.ndarray:
    """Full inputs in, full output out. Shards across 8 NeuronCores internally."""
    X = np.asarray(inputs["X"], dtype=np.float32)
    W = np.asarray(inputs["W"], dtype=np.float32)
    homo = np.asarray(inputs["homo"], dtype=np.float32)
    vertex = np.asarray(inputs["vertex"])
    edges = np.asarray(inputs["edges"])
    cfg = Cfg.from_inputs(vertex, edges)
    assert X.shape == (cfg.N, 64) and homo.shape == (cfg.E,)

    key = cfg
    if key not in _NC_CACHE:
        _NC_CACHE[key] = build_nc(cfg)
    nc = _NC_CACHE[key]

    in_maps = [prep_core_inputs(cfg, k, X, W, homo, vertex, edges)
               for k in range(cfg.n_cores)]
    res = bass_utils.run_bass_kernel_spmd(
        nc, in_maps, core_ids=list(range(cfg.n_cores)))
    out = np.concatenate(
        [res.results[k]["out"][:cfg.npc] for k in range(cfg.n_cores)], axis=0)
    return out.astype(np.float32)


# revision 28
# speedup vs baseline: 1.0469x; 1.0067x over previous
"""HyperGNN message-passing kernel v5 (nn_Conv_13778255086166) for 8 TRN2 cores.

Reference computation:
    Xp    = X @ W                                   [N, 64]
    Xe_s  = segment_sum(Xp[vertex], edges, E);  cnt = segment_sum(1, edges, E)
    Ze    = (homo / max(cnt,1)) * Xe_s              [E, 64]
    att_s = segment_sum(homo[edges], vertex, N)
    Xv    = segment_sum(Ze[edges], vertex, N) / att_s
    out   = row_l2_normalize(Xp + Xv)

Distribution: incidence lists sharded by vertex range (core k owns nodes
[k*12500, (k+1)*12500)); per-core edge partials AllReduced (bf16).

v5 vs v2 baseline (59ms -> ~1.7ms measured sustained per-exec):
  - variable per-tile slot caps (exact counts rounded to 128) instead of one
    global max cap: ~15% fewer gathered rows / masks / matmuls
  - one-hot masks built with pair-duplicated offsets so every DVE operand has
    a packed last dim -> 2x DVE mode (broadcast last-dim stride-0 disables it)
  - offsets for all tiles preloaded in one DMA (partition-major host layout)
  - pair-of-tiles processing: one idx DMA / one mask build / one psum->SBUF
    copy per pair; att column folded into the phase-2 psum (65-wide)
  - gathers ~768 idx each rotating the 4 SWDGE queues (tuned on HW)
  - staged SBUF accumulators with one strided DMA per phase; batched finalize
"""

from dataclasses import dataclass
from itertools import product

import numpy as np

import concourse.bacc as bacc
import concourse.mybir as mybir
import concourse.tile as tile
from concourse import bass_utils

F32 = mybir.dt.float32
BF16 = mybir.dt.bfloat16
I16 = mybir.dt.int16


@dataclass(frozen=True)
class Cfg:
    n_cores: int = 8
    N: int = 100000
    E: int = 25000
    caps1: tuple = ()   # per-edge-tile slot caps (multiples of 128)
    caps2: tuple = ()   # per-node-tile slot caps

    @staticmethod
    def from_inputs(vertex, edges, n_cores=8, N=100000, E=25000):
        """Exact per-tile slot caps (max over cores, rounded to 128)."""
        vertex = np.asarray(vertex).astype(np.int64)
        edges = np.asarray(edges).astype(np.int64)
        npc = N // n_cores
        etiles = ((E + 1 + 127) // 128 * 128) // 128
        ntiles = ((npc + 1 + 127) // 128 * 128) // 128
        m1 = np.zeros(etiles, np.int64)
        m2 = np.zeros(ntiles, np.int64)
        for k in range(n_cores):
            sel = (vertex >= k * npc) & (vertex < (k + 1) * npc)
            v_l, e_l = vertex[sel] - k * npc, edges[sel]
            m1 = np.maximum(m1, np.bincount(e_l >> 7, minlength=etiles))
            m2 = np.maximum(m2, np.bincount(v_l >> 7, minlength=ntiles))
        r = lambda x: int(max(128, (x + 127) // 128 * 128))
        return Cfg(n_cores=n_cores, N=N, E=E,
                   caps1=tuple(r(x) for x in m1),
                   caps2=tuple(r(x) for x in m2))

    @property
    def npc(self):
        assert self.N % self.n_cores == 0
        return self.N // self.n_cores

    @property
    def npcp(self):  # padded, with at least one spare zero row
        return (self.npc + 1 + 127) // 128 * 128

    @property
    def ntiles(self):
        return self.npcp // 128

    @property
    def ep(self):
        return (self.E + 1 + 127) // 128 * 128

    @property
    def etiles(self):
        return self.ep // 128


def _bf16():
    import ml_dtypes
    return ml_dtypes.bfloat16


def wrap_idx(idx: np.ndarray) -> np.ndarray:
    """int16 index layout for dma_gather: element j at [j%16, j//16],
    replicated across the 8 16-partition groups (one per Q7 cpu)."""
    s = idx.shape[0]
    assert s % 16 == 0
    w = np.ascontiguousarray(idx.astype(np.int16).reshape(-1, 16).T)
    return np.tile(w, (8, 1))


def prep_core_inputs(cfg: Cfg, k: int, X, W, homo, vertex, edges):
    """Host-side shard/sort/pad for core k (index/layout reorganization only)."""
    bf16 = _bf16()
    npc, npcp = cfg.npc, cfg.npcp
    vertex = np.asarray(vertex)
    edges = np.asarray(edges)
    sel = (vertex >= k * npc) & (vertex < (k + 1) * npc)
    v_l = (vertex[sel] - k * npc).astype(np.int64)
    e_l = edges[sel].astype(np.int64)

    def build(seg, other, tiles_n, caps, pad_gather):
        caps = np.asarray(caps)
        o = np.argsort(seg, kind="stable")
        s, g = seg[o], other[o]
        t_of = s >> 7
        counts = np.bincount(t_of, minlength=tiles_n)
        assert (counts <= caps).all(), (counts.max(), caps.max())
        starts = np.cumsum(counts) - counts
        capoff = np.cumsum(caps) - caps
        rank = np.arange(len(s)) - starts[t_of]
        dest = capoff[t_of] + rank
        S = int(caps.sum())
        gi = np.full(S, pad_gather, np.int64)
        off = np.zeros(S, np.float32)
        gi[dest] = g
        off[dest] = (s & 127).astype(np.float32)
        # flat idx blocks (pair-of-tiles contiguous, partition-major within
        # a pair), partition-major paired offsets
        wraps = [wrap_idx(gi[capoff[t]:capoff[t] + caps[t]])
                 for t in range(tiles_n)]
        idx_flat = np.concatenate(
            [np.hstack(wraps[q:q + 2]).ravel()
             for q in range(0, tiles_n, 2)])
        offp = np.concatenate(
            [off[capoff[t]:capoff[t] + caps[t]].reshape(caps[t] // 128, 128).T
             for t in range(tiles_n)], axis=1)          # [128, sum_c]
        off_pm = np.repeat(offp, 2, axis=1).astype(bf16)  # [128, sum_c*2]
        return idx_flat, np.ascontiguousarray(off_pm)

    # P1: segment by edge, gather by local vertex; pads gather zero row npc.
    g1, off1 = build(e_l, v_l, cfg.etiles, cfg.caps1, pad_gather=npc)
    # P2: segment by local vertex, gather by edge; pads gather zero row E.
    g2, off2 = build(v_l, e_l, cfg.ntiles, cfg.caps2, pad_gather=cfg.E)

    Xt = np.zeros((64, npcp), np.float32)
    Xt[:, :npc] = np.asarray(X)[k * npc:(k + 1) * npc].T

    homo_pad = np.zeros(cfg.ep, np.float32)
    homo_pad[:cfg.E] = np.asarray(homo)
    homo_t = np.ascontiguousarray(homo_pad.reshape(cfg.etiles, 128).T)

    # global per-edge incidence counts: pure index data -> host computes
    cnt = np.bincount(edges.astype(np.int64), minlength=cfg.E).astype(np.float32)
    cntr_pad = np.zeros(cfg.ep, np.float32)
    cntr_pad[:cfg.E] = 1.0 / np.maximum(cnt, 1.0)
    cntr_t = np.ascontiguousarray(cntr_pad.reshape(cfg.etiles, 128).T)

    iota = np.broadcast_to(np.arange(128, dtype=np.float32),
                           (128, 128)).astype(bf16).copy()

    return {
        "Xt": Xt,
        "W": np.asarray(W, dtype=np.float32),
        "homo_t": homo_t,
        "cntr_t": cntr_t,
        "iota": iota,
        "g1": g1,
        "off1": off1,
        "g2": g2,
        "off2": off2,
    }


def build_nc(cfg: Cfg, for_sim: bool = False, variant: str = "full",
             repeat: int = 1, gsplit: int = 6):
    """variant: full | nocc | p0 | p1 | p2 | p1n | p2n | nog"""
    no_g1 = variant in ("p1n", "nog")
    no_g2 = variant in ("p2n", "nog")
    variant = {"p1n": "p1", "p2n": "p2", "nog": "nocc"}.get(variant, variant)
    caps1, caps2 = cfg.caps1, cfg.caps2
    cs1 = [c // 128 for c in caps1]
    cs2 = [c // 128 for c in caps2]
    c1m, c2m = max(cs1), max(cs2)
    co1 = np.cumsum([0] + cs1)   # per-tile column offsets (128-slot units)
    co2 = np.cumsum([0] + cs2)
    sum1, sum2 = int(co1[-1]), int(co2[-1])
    nt, et = cfg.ntiles, cfg.etiles
    nc = bacc.Bacc("TRN2", target_bir_lowering=False, debug=False,
                   num_devices=1 if for_sim else cfg.n_cores,
                   num_swdge_queues=4, dynamic_dma_scratch_size=32768)

    xt_d = nc.dram_tensor("Xt", [64, cfg.npcp], F32, kind="ExternalInput")
    w_d = nc.dram_tensor("W", [64, 64], F32, kind="ExternalInput")
    homo_d = nc.dram_tensor("homo_t", [128, et], F32, kind="ExternalInput")
    cntr_d = nc.dram_tensor("cntr_t", [128, et], F32, kind="ExternalInput")
    iota_d = nc.dram_tensor("iota", [128, 128], BF16, kind="ExternalInput")
    g1_d = nc.dram_tensor("g1", [sum1 * 1024], I16, kind="ExternalInput")
    off1_d = nc.dram_tensor("off1", [128, sum1 * 2], BF16, kind="ExternalInput")
    g2_d = nc.dram_tensor("g2", [sum2 * 1024], I16, kind="ExternalInput")
    off2_d = nc.dram_tensor("off2", [128, sum2 * 2], BF16, kind="ExternalInput")
    out_d = nc.dram_tensor("out", [cfg.npcp, 64], F32, kind="ExternalOutput")

    xp_d = nc.dram_tensor("XpD", [cfg.npcp, 128], BF16, kind="Internal")
    eacc_d = nc.dram_tensor("EaccD", [cfg.ep, 64], BF16, kind="Internal")
    ered_d = nc.dram_tensor("EredD", [cfg.ep, 64], BF16, kind="Internal",
                            addr_space="Shared")
    zef_d = nc.dram_tensor("ZeFD", [cfg.ep, 128], BF16, kind="Internal")

    with tile.TileContext(nc) as tc:
        xtiles = next(d for d in range(min(7, nt), 0, -1) if nt % d == 0)
        xchunk = xtiles * 128  # phase-0 X streamed in nt/xtiles chunks
        zchunk = next(d for d in range(min(7, et), 0, -1) if et % d == 0)
        fchunk = next(d for d in range(min(14, nt), 0, -1) if nt % d == 0)
        with (
            tc.tile_pool(name="const", bufs=1) as pc,
            tc.tile_pool(name="xin", bufs=2) as px,
            tc.tile_pool(name="idx", bufs=8) as pidx,
            tc.tile_pool(name="gather", bufs=4) as pg,
            tc.tile_pool(name="onehot", bufs=3) as pm,
            tc.tile_pool(name="ze", bufs=2) as pz,
            tc.tile_pool(name="fin", bufs=2) as pf,
            tc.tile_pool(name="psum", bufs=4, space="PSUM") as pp,
        ):
            w_sb = pc.tile([64, 64], F32)
            nc.sync.dma_start(out=w_sb[:], in_=w_d[:])
            iota_sb = pc.tile([128, 64, 2], BF16)
            nc.sync.dma_start(
                out=iota_sb[:],
                in_=iota_d[:].rearrange("p (a b) -> p a b", b=2))
            homo_sb = pc.tile([128, et], F32)
            nc.sync.dma_start(out=homo_sb[:], in_=homo_d[:])
            cntr_sb = pc.tile([128, et], F32)
            nc.sync.dma_start(out=cntr_sb[:], in_=cntr_d[:])
            off1_sb = pc.tile([128, sum1, 2], BF16)
            nc.scalar.dma_start(
                out=off1_sb[:],
                in_=off1_d[:].rearrange("p (a b) -> p a b", b=2))
            off2_sb = pc.tile([128, sum2, 2], BF16)
            nc.scalar.dma_start(
                out=off2_sb[:],
                in_=off2_d[:].rearrange("p (a b) -> p a b", b=2))

            # persistent SBUF stages
            xp_all = pc.tile([128, nt, 64], F32)    # f32 Xp for phase-2 add
            acc_all = pc.tile([128, et, 64], BF16)  # phase-1 edge partials
            o_all = pc.tile([128, nt, 65], F32)     # phase-2 sums (col 64=att)

            # phase 0: Xp = X_local @ W -> xp_all (SBUF) and xp_d (DRAM, bf16)
            xp_view = xp_d.ap().rearrange("(t p) f -> p t f", p=128)
            for rep, cs in product(range(repeat), range(0, cfg.npcp, xchunk)):
                xc = px.tile([64, xchunk], F32, tag="xc")
                nc.sync.dma_start(out=xc[:], in_=xt_d[:, cs:cs + xchunk])
                xq = px.tile([128, xtiles, 64], BF16, tag="xq")
                for u in range(xtiles):
                    t = cs // 128 + u
                    ps = pp.tile([128, 65], F32, tag="ps", bufs=2)
                    nc.tensor.matmul(ps[:, 0:64], lhsT=xc[:, u * 128:(u + 1) * 128],
                                     rhs=w_sb[:], start=True, stop=True)
                    nc.vector.tensor_copy(out=xp_all[:, t, :], in_=ps[:, 0:64])
                    nc.scalar.copy(out=xq[:, u, :], in_=ps[:, 0:64])
                nc.sync.dma_start(
                    out=xp_view[:, cs // 128:cs // 128 + xtiles, 0:64], in_=xq[:])

            # phases 1/2: pair-of-tiles granularity. One idx DMA, one mask
            # is_equal, ~768-idx gathers (crossing tile boundaries), one
            # psum->SBUF copy per pair. Pipelined one pair ahead so the
            # in-order DVE/Q7 queues stay ahead of the PE matmul stream.
            LA = 1
            p1_reps = 0 if variant in ("p0", "p2") else repeat
            qctr = [0]

            def stage_pair(q, cs, co, pw, src_d, idx_d, off_sb, gtag, ctag,
                           no_g):
                t0 = 2 * q
                tiles = list(range(t0, min(t0 + 2, len(cs))))
                o = int(co[t0])
                C = sum(cs[t] for t in tiles)
                gi = pidx.tile([128, 8 * pw], I16, tag="i" + gtag)
                nc.sync.dma_start(
                    out=gi[:, 0:8 * C],
                    in_=idx_d[o * 1024:(o + C) * 1024]
                        .rearrange("(p c) -> p c", p=128))
                g = pg.tile([128, pw, 128], BF16, tag=gtag, bufs=2)
                if no_g:
                    nc.vector.memset(g[:, 0:C, :], 0.0)
                else:
                    nsp = max(1, (C + gsplit - 1) // gsplit)
                    for h in range(nsp):
                        a = (h * C) // nsp
                        b = ((h + 1) * C) // nsp
                        if a == b:
                            continue
                        nc.gpsimd.dma_gather(
                            g[:, a:b, :], src_d[:], gi[:, a * 8:b * 8],
                            (b - a) * 128, (b - a) * 128, 128,
                            single_packet=False,
                            queue_num=qctr[0] % 4)
                        qctr[0] += 1
                mt = pm.tile([128, pw, 64, 2], BF16, tag=ctag, bufs=2)
                nc.vector.tensor_tensor(
                    out=mt[:, 0:C, :, :],
                    in0=iota_sb[:].unsqueeze(1).broadcast_to([128, C, 64, 2]),
                    in1=off_sb[:, o:o + C, :].unsqueeze(2)
                        .broadcast_to([128, C, 64, 2]),
                    op=mybir.AluOpType.is_equal)
                return g, mt

            def run_phase(cs, co, pw, src_d, idx_d, off_sb, gtag, ctag, no_g,
                          fcols, sink):
                ntl = len(cs)
                nq = (ntl + 1) // 2

                def stage(q):
                    return stage_pair(q, cs, co, pw, src_d, idx_d, off_sb,
                                      gtag, ctag, no_g)

                pend = {q: stage(q) for q in range(min(LA + 1, nq))}
                for q in range(nq):
                    if q + LA + 1 < nq:
                        pend[q + LA + 1] = stage(q + LA + 1)
                    g, mt = pend.pop(q)
                    tiles = list(range(2 * q, min(2 * q + 2, ntl)))
                    ps = pp.tile([128, 2, 65], F32, tag="ps2t")
                    jj = 0
                    for i, t in enumerate(tiles):
                        for j in range(cs[t]):
                            nc.tensor.matmul(
                                ps[:, i, 0:fcols],
                                lhsT=mt[:, jj, :, :]
                                    .rearrange("p a b -> p (a b)"),
                                rhs=g[:, jj, 0:fcols],
                                start=(j == 0), stop=(j == cs[t] - 1))
                            jj += 1
                    sink(ps, tiles)

            def sink1(ps, tiles):
                nc.vector.tensor_copy(
                    out=acc_all[:, tiles[0]:tiles[0] + len(tiles), :],
                    in_=ps[:, 0:len(tiles), 0:64])

            for rep in range(p1_reps):
                run_phase(cs1, co1, 2 * c1m, xp_d, g1_d, off1_sb, "g1", "mt1",
                          no_g1, 64, sink1)
            eacc_view = eacc_d.ap().rearrange("(t p) f -> p t f", p=128)
            for rep in range(p1_reps):
                nc.sync.dma_start(out=eacc_view[:], in_=acc_all[:])

            # AllReduce edge partials (bf16)
            cc_reps = 0 if variant in ("p0", "p1") else repeat
            for rep in range(cc_reps):
                if for_sim or variant in ("nocc", "p2"):
                    nc.sync.dma_start(out=ered_d[:], in_=eacc_d[:])
                else:
                    nc.gpsimd.collective_compute(
                        "AllReduce", mybir.AluOpType.add,
                        replica_groups=[list(range(cfg.n_cores))],
                        ins=[eacc_d.ap()], outs=[ered_d.ap()],
                    )

            # Ze build: zef rows = [Ye*homo | homo | junk], chunk-batched
            scale_sb = pc.tile([128, et], F32)
            nc.vector.tensor_tensor(out=scale_sb[:], in0=homo_sb[:],
                                    in1=cntr_sb[:], op=mybir.AluOpType.mult)
            er_view = ered_d.ap().rearrange("(t p) f -> p t f", p=128)
            zf_view = zef_d.ap().rearrange("(t p) f -> p t f", p=128)
            for rep, zs in product(range(cc_reps), range(0, et, zchunk)):
                er = pz.tile([128, zchunk, 64], BF16, tag="er")
                nc.sync.dma_start(out=er[:], in_=er_view[:, zs:zs + zchunk, :])
                z = pz.tile([128, zchunk, 128], BF16, tag="z")
                nc.vector.tensor_tensor(
                    out=z[:, :, 0:64], in0=er[:],
                    in1=scale_sb[:, zs:zs + zchunk].unsqueeze(2)
                        .broadcast_to([128, zchunk, 64]),
                    op=mybir.AluOpType.mult)
                nc.vector.tensor_copy(
                    out=z[:, :, 64:65],
                    in_=homo_sb[:, zs:zs + zchunk].unsqueeze(2))
                nc.sync.dma_start(out=zf_view[:, zs:zs + zchunk, :], in_=z[:])

            # phase 2: node-tile accumulation (same pairing as phase 1)
            def sink2(ps, tiles):
                nc.vector.tensor_copy(
                    out=o_all[:, tiles[0]:tiles[0] + len(tiles), :],
                    in_=ps[:, 0:len(tiles), :])

            for rep in range(cc_reps):
                run_phase(cs2, co2, 2 * c2m, zef_d, g2_d, off2_sb, "g2", "mt2",
                          no_g2, 65, sink2)

            # batched finalize, fchunk node tiles at a time
            out_view = out_d.ap().rearrange("(t p) f -> p t f", p=128)
            for rep, fs in product(range(cc_reps), range(0, nt, fchunk)):
                fe = fs + fchunk
                attm = pf.tile([128, fchunk, 1], F32, tag="attm", bufs=1)
                nc.vector.tensor_scalar_max(out=attm[:],
                                            in0=o_all[:, fs:fe, 64:65],
                                            scalar1=1e-30)
                arec = pf.tile([128, fchunk, 1], F32, tag="arec", bufs=1)
                nc.vector.reciprocal(out=arec[:], in_=attm[:])
                o3 = pf.tile([128, fchunk, 64], F32, tag="o3", bufs=1)
                nc.vector.tensor_tensor(
                    out=o3[:], in0=o_all[:, fs:fe, 0:64],
                    in1=arec[:].broadcast_to([128, fchunk, 64]),
                    op=mybir.AluOpType.mult)
                nc.vector.tensor_tensor(out=o3[:], in0=o3[:],
                                        in1=xp_all[:, fs:fe, :],
                                        op=mybir.AluOpType.add)
                sq = pf.tile([128, fchunk, 64], F32, tag="sq", bufs=1)
                nc.vector.tensor_tensor(out=sq[:], in0=o3[:], in1=o3[:],
                                        op=mybir.AluOpType.mult)
                rs = pf.tile([128, fchunk, 1], F32, tag="rs", bufs=1)
                nc.vector.reduce_sum(out=rs[:], in_=sq[:],
                                     axis=mybir.AxisListType.X)
                rn = pf.tile([128, fchunk, 1], F32, tag="rn", bufs=1)
                nc.scalar.sqrt(out=rn[:], in_=rs[:])
                rnm = pf.tile([128, fchunk, 1], F32, tag="rnm", bufs=1)
                nc.vector.tensor_scalar_max(out=rnm[:], in0=rn[:], scalar1=1e-30)
                rrec = pf.tile([128, fchunk, 1], F32, tag="rrec", bufs=1)
                nc.vector.reciprocal(out=rrec[:], in_=rnm[:])
                nc.vector.tensor_tensor(
                    out=o3[:], in0=o3[:],
                    in1=rrec[:].broadcast_to([128, fchunk, 64]),
                    op=mybir.AluOpType.mult)
                nc.sync.dma_start(out=out_view[:, fs:fe, :], in_=o3[:])

    nc.compile()
    return nc


_NC_CACHE = {}


def kernel(**inputs) -> np.ndarray:
    """Full inputs in, full output out. Shards across 8 NeuronCores internally."""
    X = np.asarray(inputs["X"], dtype=np.float32)
    W = np.asarray(inputs["W"], dtype=np.float32)
    homo = np.asarray(inputs["homo"], dtype=np.float32)
    vertex = np.asarray(inputs["vertex"])
    edges = np.asarray(inputs["edges"])
    cfg = Cfg.from_inputs(vertex, edges)
    assert X.shape == (cfg.N, 64) and homo.shape == (cfg.E,)

    key = cfg
    if key not in _NC_CACHE:
        _NC_CACHE[key] = build_nc(cfg)
    nc = _NC_CACHE[key]

    in_maps = [prep_core_inputs(cfg, k, X, W, homo, vertex, edges)
               for k in range(cfg.n_cores)]
    res = bass_utils.run_bass_kernel_spmd(
        nc, in_maps, core_ids=list(range(cfg.n_cores)))
    out = np.concatenate(
        [res.results[k]["out"][:cfg.npc] for k in range(cfg.n_cores)], axis=0)
    return out.astype(np.float32)


# revision 30
# speedup vs baseline: 1.0575x; 1.0102x over previous
"""HyperGNN message-passing kernel v5 (nn_Conv_13778255086166) for 8 TRN2 cores.

Reference computation:
    Xp    = X @ W                                   [N, 64]
    Xe_s  = segment_sum(Xp[vertex], edges, E);  cnt = segment_sum(1, edges, E)
    Ze    = (homo / max(cnt,1)) * Xe_s              [E, 64]
    att_s = segment_sum(homo[edges], vertex, N)
    Xv    = segment_sum(Ze[edges], vertex, N) / att_s
    out   = row_l2_normalize(Xp + Xv)

Distribution: incidence lists sharded by vertex range (core k owns nodes
[k*12500, (k+1)*12500)); per-core edge partials AllReduced (bf16).

v5 vs v2 baseline (59ms -> ~1.7ms measured sustained per-exec):
  - variable per-tile slot caps (exact counts rounded to 128) instead of one
    global max cap: ~15% fewer gathered rows / masks / matmuls
  - one-hot masks built with pair-duplicated offsets so every DVE operand has
    a packed last dim -> 2x DVE mode (broadcast last-dim stride-0 disables it)
  - offsets for all tiles preloaded in one DMA (partition-major host layout)
  - pair-of-tiles processing: one idx DMA / one mask build / one psum->SBUF
    copy per pair; att column folded into the phase-2 psum (65-wide)
  - gathers ~768 idx each rotating the 4 SWDGE queues (tuned on HW)
  - staged SBUF accumulators with one strided DMA per phase; batched finalize
"""

from dataclasses import dataclass
from itertools import product

import numpy as np

import concourse.bacc as bacc
import concourse.mybir as mybir
import concourse.tile as tile
from concourse import bass_utils

F32 = mybir.dt.float32
BF16 = mybir.dt.bfloat16
I16 = mybir.dt.int16


@dataclass(frozen=True)
class Cfg:
    n_cores: int = 8
    N: int = 100000
    E: int = 25000
    caps1: tuple = ()   # per-edge-tile slot caps (multiples of 128)
    caps2: tuple = ()   # per-node-tile slot caps

    @staticmethod
    def from_inputs(vertex, edges, n_cores=8, N=100000, E=25000):
        """Exact per-tile slot caps (max over cores, rounded to 128)."""
        vertex = np.asarray(vertex).astype(np.int64)
        edges = np.asarray(edges).astype(np.int64)
        npc = N // n_cores
        etiles = ((E + 1 + 127) // 128 * 128) // 128
        ntiles = ((npc + 1 + 127) // 128 * 128) // 128
        m1 = np.zeros(etiles, np.int64)
        m2 = np.zeros(ntiles, np.int64)
        for k in range(n_cores):
            sel = (vertex >= k * npc) & (vertex < (k + 1) * npc)
            v_l, e_l = vertex[sel] - k * npc, edges[sel]
            m1 = np.maximum(m1, np.bincount(e_l >> 7, minlength=etiles))
            m2 = np.maximum(m2, np.bincount(v_l >> 7, minlength=ntiles))
        r = lambda x: int(max(128, (x + 127) // 128 * 128))
        return Cfg(n_cores=n_cores, N=N, E=E,
                   caps1=tuple(r(x) for x in m1),
                   caps2=tuple(r(x) for x in m2))

    @property
    def npc(self):
        assert self.N % self.n_cores == 0
        return self.N // self.n_cores

    @property
    def npcp(self):  # padded, with at least one spare zero row
        return (self.npc + 1 + 127) // 128 * 128

    @property
    def ntiles(self):
        return self.npcp // 128

    @property
    def ep(self):
        return (self.E + 1 + 127) // 128 * 128

    @property
    def etiles(self):
        return self.ep // 128


def _bf16():
    import ml_dtypes
    return ml_dtypes.bfloat16


def wrap_idx(idx: np.ndarray) -> np.ndarray:
    """int16 index layout for dma_gather: element j at [j%16, j//16],
    replicated across the 8 16-partition groups (one per Q7 cpu)."""
    s = idx.shape[0]
    assert s % 16 == 0
    w = np.ascontiguousarray(idx.astype(np.int16).reshape(-1, 16).T)
    return np.tile(w, (8, 1))


def prep_core_inputs(cfg: Cfg, k: int, X, W, homo, vertex, edges):
    """Host-side shard/sort/pad for core k (index/layout reorganization only)."""
    bf16 = _bf16()
    npc, npcp = cfg.npc, cfg.npcp
    vertex = np.asarray(vertex)
    edges = np.asarray(edges)
    sel = (vertex >= k * npc) & (vertex < (k + 1) * npc)
    v_l = (vertex[sel] - k * npc).astype(np.int64)
    e_l = edges[sel].astype(np.int64)

    def build(seg, other, tiles_n, caps, pad_gather):
        caps = np.asarray(caps)
        o = np.argsort(seg, kind="stable")
        s, g = seg[o], other[o]
        t_of = s >> 7
        counts = np.bincount(t_of, minlength=tiles_n)
        assert (counts <= caps).all(), (counts.max(), caps.max())
        starts = np.cumsum(counts) - counts
        capoff = np.cumsum(caps) - caps
        rank = np.arange(len(s)) - starts[t_of]
        dest = capoff[t_of] + rank
        S = int(caps.sum())
        gi = np.full(S, pad_gather, np.int64)
        off = np.zeros(S, np.float32)
        gi[dest] = g
        off[dest] = (s & 127).astype(np.float32)
        # flat idx blocks (pair-of-tiles contiguous, partition-major within
        # a pair), partition-major paired offsets
        wraps = [wrap_idx(gi[capoff[t]:capoff[t] + caps[t]])
                 for t in range(tiles_n)]
        idx_flat = np.concatenate(
            [np.hstack(wraps[q:q + 2]).ravel()
             for q in range(0, tiles_n, 2)])
        offp = np.concatenate(
            [off[capoff[t]:capoff[t] + caps[t]].reshape(caps[t] // 128, 128).T
             for t in range(tiles_n)], axis=1)          # [128, sum_c]
        off_pm = np.repeat(offp, 2, axis=1).astype(bf16)  # [128, sum_c*2]
        return idx_flat, np.ascontiguousarray(off_pm)

    # P1: segment by edge, gather by local vertex; pads gather zero row npc.
    g1, off1 = build(e_l, v_l, cfg.etiles, cfg.caps1, pad_gather=npc)
    # P2: segment by local vertex, gather by edge; pads gather zero row E.
    g2, off2 = build(v_l, e_l, cfg.ntiles, cfg.caps2, pad_gather=cfg.E)

    Xt = np.zeros((64, npcp), np.float32)
    Xt[:, :npc] = np.asarray(X)[k * npc:(k + 1) * npc].T

    homo_pad = np.zeros(cfg.ep, np.float32)
    homo_pad[:cfg.E] = np.asarray(homo)
    homo_t = np.ascontiguousarray(homo_pad.reshape(cfg.etiles, 128).T)

    # global per-edge incidence counts: pure index data -> host computes
    cnt = np.bincount(edges.astype(np.int64), minlength=cfg.E).astype(np.float32)
    cntr_pad = np.zeros(cfg.ep, np.float32)
    cntr_pad[:cfg.E] = 1.0 / np.maximum(cnt, 1.0)
    cntr_t = np.ascontiguousarray(cntr_pad.reshape(cfg.etiles, 128).T)

    iota = np.broadcast_to(np.arange(128, dtype=np.float32),
                           (128, 128)).astype(bf16).copy()

    return {
        "Xt": Xt,
        "W": np.asarray(W, dtype=np.float32),
        "homo_t": homo_t,
        "cntr_t": cntr_t,
        "iota": iota,
        "g1": g1,
        "off1": off1,
        "g2": g2,
        "off2": off2,
    }


def build_nc(cfg: Cfg, for_sim: bool = False, variant: str = "full",
             repeat: int = 1, gsplit: int = 6):
    """variant: full | nocc | p0 | p1 | p2 | p1n | p2n | nog"""
    no_g1 = variant in ("p1n", "nog")
    no_g2 = variant in ("p2n", "nog")
    variant = {"p1n": "p1", "p2n": "p2", "nog": "nocc"}.get(variant, variant)
    caps1, caps2 = cfg.caps1, cfg.caps2
    cs1 = [c // 128 for c in caps1]
    cs2 = [c // 128 for c in caps2]
    c1m, c2m = max(cs1), max(cs2)
    co1 = np.cumsum([0] + cs1)   # per-tile column offsets (128-slot units)
    co2 = np.cumsum([0] + cs2)
    sum1, sum2 = int(co1[-1]), int(co2[-1])
    nt, et = cfg.ntiles, cfg.etiles
    nc = bacc.Bacc("TRN2", target_bir_lowering=False, debug=False,
                   num_devices=1 if for_sim else cfg.n_cores,
                   num_swdge_queues=4, dynamic_dma_scratch_size=32768)

    xt_d = nc.dram_tensor("Xt", [64, cfg.npcp], F32, kind="ExternalInput")
    w_d = nc.dram_tensor("W", [64, 64], F32, kind="ExternalInput")
    homo_d = nc.dram_tensor("homo_t", [128, et], F32, kind="ExternalInput")
    cntr_d = nc.dram_tensor("cntr_t", [128, et], F32, kind="ExternalInput")
    iota_d = nc.dram_tensor("iota", [128, 128], BF16, kind="ExternalInput")
    g1_d = nc.dram_tensor("g1", [sum1 * 1024], I16, kind="ExternalInput")
    off1_d = nc.dram_tensor("off1", [128, sum1 * 2], BF16, kind="ExternalInput")
    g2_d = nc.dram_tensor("g2", [sum2 * 1024], I16, kind="ExternalInput")
    off2_d = nc.dram_tensor("off2", [128, sum2 * 2], BF16, kind="ExternalInput")
    out_d = nc.dram_tensor("out", [cfg.npcp, 64], F32, kind="ExternalOutput")

    xp_d = nc.dram_tensor("XpD", [cfg.npcp, 128], BF16, kind="Internal")
    eacc_d = nc.dram_tensor("EaccD", [cfg.ep, 64], BF16, kind="Internal")
    ered_d = nc.dram_tensor("EredD", [cfg.ep, 64], BF16, kind="Internal",
                            addr_space="Shared")
    zef_d = nc.dram_tensor("ZeFD", [cfg.ep, 128], BF16, kind="Internal")

    with tile.TileContext(nc) as tc:
        xtiles = next(d for d in range(min(7, nt), 0, -1) if nt % d == 0)
        xchunk = xtiles * 128  # phase-0 X streamed in nt/xtiles chunks
        zchunk = next(d for d in range(min(7, et), 0, -1) if et % d == 0)
        fchunk = next(d for d in range(min(14, nt), 0, -1) if nt % d == 0)
        with (
            tc.tile_pool(name="const", bufs=1) as pc,
            tc.tile_pool(name="xin", bufs=2) as px,
            tc.tile_pool(name="idx", bufs=8) as pidx,
            tc.tile_pool(name="gather", bufs=4) as pg,
            tc.tile_pool(name="onehot", bufs=3) as pm,
            tc.tile_pool(name="ze", bufs=2) as pz,
            tc.tile_pool(name="fin", bufs=2) as pf,
            tc.tile_pool(name="psum", bufs=4, space="PSUM") as pp,
        ):
            w_sb = pc.tile([64, 64], F32)
            nc.sync.dma_start(out=w_sb[:], in_=w_d[:])
            iota_sb = pc.tile([128, 64, 2], BF16)
            nc.sync.dma_start(
                out=iota_sb[:],
                in_=iota_d[:].rearrange("p (a b) -> p a b", b=2))
            homo_sb = pc.tile([128, et], F32)
            nc.sync.dma_start(out=homo_sb[:], in_=homo_d[:])
            cntr_sb = pc.tile([128, et], F32)
            nc.sync.dma_start(out=cntr_sb[:], in_=cntr_d[:])
            off1_sb = pc.tile([128, sum1, 2], BF16)
            nc.scalar.dma_start(
                out=off1_sb[:],
                in_=off1_d[:].rearrange("p (a b) -> p a b", b=2))
            off2_sb = pc.tile([128, sum2, 2], BF16)
            nc.scalar.dma_start(
                out=off2_sb[:],
                in_=off2_d[:].rearrange("p (a b) -> p a b", b=2))

            # persistent SBUF stages
            xp_all = pc.tile([128, nt, 64], F32)    # f32 Xp for phase-2 add
            acc_all = pc.tile([128, et, 64], BF16)  # phase-1 edge partials
            o_all = pc.tile([128, nt, 65], F32)     # phase-2 sums (col 64=att)

            # phase 0: Xp = X_local @ W -> xp_all (SBUF) and xp_d (DRAM, bf16)
            xp_view = xp_d.ap().rearrange("(t p) f -> p t f", p=128)
            for rep, cs in product(range(repeat), range(0, cfg.npcp, xchunk)):
                xc = px.tile([64, xchunk], F32, tag="xc")
                nc.sync.dma_start(out=xc[:], in_=xt_d[:, cs:cs + xchunk])
                xq = px.tile([128, xtiles, 64], BF16, tag="xq")
                for u in range(xtiles):
                    t = cs // 128 + u
                    ps = pp.tile([128, 65], F32, tag="ps", bufs=2)
                    nc.tensor.matmul(ps[:, 0:64], lhsT=xc[:, u * 128:(u + 1) * 128],
                                     rhs=w_sb[:], start=True, stop=True)
                    nc.vector.tensor_copy(out=xp_all[:, t, :], in_=ps[:, 0:64])
                    nc.scalar.copy(out=xq[:, u, :], in_=ps[:, 0:64])
                nc.sync.dma_start(
                    out=xp_view[:, cs // 128:cs // 128 + xtiles, 0:64], in_=xq[:])

            # phases 1/2: pair-of-tiles granularity. One idx DMA, one mask
            # is_equal, ~768-idx gathers (crossing tile boundaries), one
            # psum->SBUF copy per pair. Pipelined one pair ahead so the
            # in-order DVE/Q7 queues stay ahead of the PE matmul stream.
            LA = 1
            p1_reps = 0 if variant in ("p0", "p2") else repeat
            qctr = [0]

            def stage_pair(q, cs, co, pw, src_d, idx_d, off_sb, gtag, ctag,
                           no_g):
                t0 = 2 * q
                tiles = list(range(t0, min(t0 + 2, len(cs))))
                o = int(co[t0])
                C = sum(cs[t] for t in tiles)
                gi = pidx.tile([128, 8 * pw], I16, tag="i" + gtag)
                nc.sync.dma_start(
                    out=gi[:, 0:8 * C],
                    in_=idx_d[o * 1024:(o + C) * 1024]
                        .rearrange("(p c) -> p c", p=128))
                g = pg.tile([128, pw, 128], BF16, tag=gtag, bufs=2)
                if no_g:
                    nc.vector.memset(g[:, 0:C, :], 0.0)
                else:
                    nsp = max(1, (C + gsplit - 1) // gsplit)
                    for h in range(nsp):
                        a = (h * C) // nsp
                        b = ((h + 1) * C) // nsp
                        if a == b:
                            continue
                        nc.gpsimd.dma_gather(
                            g[:, a:b, :], src_d[:], gi[:, a * 8:b * 8],
                            (b - a) * 128, (b - a) * 128, 128,
                            single_packet=False,
                            queue_num=qctr[0] % 4)
                        qctr[0] += 1
                mt = pm.tile([128, pw, 64, 2], BF16, tag=ctag, bufs=2)
                nc.vector.tensor_tensor(
                    out=mt[:, 0:C, :, :],
                    in0=iota_sb[:].unsqueeze(1).broadcast_to([128, C, 64, 2]),
                    in1=off_sb[:, o:o + C, :].unsqueeze(2)
                        .broadcast_to([128, C, 64, 2]),
                    op=mybir.AluOpType.is_equal)
                return g, mt

            def run_phase(cs, co, pw, src_d, idx_d, off_sb, gtag, ctag, no_g,
                          fcols, sink):
                ntl = len(cs)
                nq = (ntl + 1) // 2

                def stage(q):
                    return stage_pair(q, cs, co, pw, src_d, idx_d, off_sb,
                                      gtag, ctag, no_g)

                pend = {q: stage(q) for q in range(min(LA + 1, nq))}
                for q in range(nq):
                    if q + LA + 1 < nq:
                        pend[q + LA + 1] = stage(q + LA + 1)
                    g, mt = pend.pop(q)
                    tiles = list(range(2 * q, min(2 * q + 2, ntl)))
                    ps = pp.tile([128, 2, 65], F32, tag="ps2t")
                    jj = 0
                    for i, t in enumerate(tiles):
                        for j in range(cs[t]):
                            nc.tensor.matmul(
                                ps[:, i, 0:fcols],
                                lhsT=mt[:, jj, :, :]
                                    .rearrange("p a b -> p (a b)"),
                                rhs=g[:, jj, 0:fcols],
                                start=(j == 0), stop=(j == cs[t] - 1))
                            jj += 1
                    sink(ps, tiles)

            def sink1(ps, tiles):
                nc.vector.tensor_copy(
                    out=acc_all[:, tiles[0]:tiles[0] + len(tiles), :],
                    in_=ps[:, 0:len(tiles), 0:64])

            for rep in range(p1_reps):
                run_phase(cs1, co1, 2 * c1m, xp_d, g1_d, off1_sb, "g1", "mt1",
                          no_g1, 64, sink1)
            eacc_view = eacc_d.ap().rearrange("(t p) f -> p t f", p=128)
            for rep in range(p1_reps):
                nc.sync.dma_start(out=eacc_view[:], in_=acc_all[:])

            # AllReduce edge partials (bf16)
            cc_reps = 0 if variant in ("p0", "p1") else repeat
            for rep in range(cc_reps):
                if for_sim or variant in ("nocc", "p2"):
                    nc.sync.dma_start(out=ered_d[:], in_=eacc_d[:])
                else:
                    nc.gpsimd.collective_compute(
                        "AllReduce", mybir.AluOpType.add,
                        replica_groups=[list(range(cfg.n_cores))],
                        ins=[eacc_d.ap()], outs=[ered_d.ap()],
                    )

            # Ze build: zef rows = [Ye*homo | homo | junk], chunk-batched
            scale_sb = pc.tile([128, et], F32)
            nc.vector.tensor_tensor(out=scale_sb[:], in0=homo_sb[:],
                                    in1=cntr_sb[:], op=mybir.AluOpType.mult)
            er_view = ered_d.ap().rearrange("(t p) f -> p t f", p=128)
            zf_view = zef_d.ap().rearrange("(t p) f -> p t f", p=128)
            for rep, zs in product(range(cc_reps), range(0, et, zchunk)):
                er = pz.tile([128, zchunk, 64], BF16, tag="er")
                nc.sync.dma_start(out=er[:], in_=er_view[:, zs:zs + zchunk, :])
                z = pz.tile([128, zchunk, 128], BF16, tag="z")
                nc.vector.tensor_tensor(
                    out=z[:, :, 0:64], in0=er[:],
                    in1=scale_sb[:, zs:zs + zchunk].unsqueeze(2)
                        .broadcast_to([128, zchunk, 64]),
                    op=mybir.AluOpType.mult)
                nc.vector.tensor_copy(
                    out=z[:, :, 64:65],
                    in_=homo_sb[:, zs:zs + zchunk].unsqueeze(2))
                nc.sync.dma_start(out=zf_view[:, zs:zs + zchunk, :], in_=z[:])

            # phase 2: node-tile accumulation (same pairing as phase 1)
            def sink2(ps, tiles):
                nc.vector.tensor_copy(
                    out=o_all[:, tiles[0]:tiles[0] + len(tiles), :],
                    in_=ps[:, 0:len(tiles), :])

            for rep in range(cc_reps):
                run_phase(cs2, co2, 2 * c2m, zef_d, g2_d, off2_sb, "g2", "mt2",
                          no_g2, 65, sink2)

            # batched finalize, fchunk node tiles at a time
            out_view = out_d.ap().rearrange("(t p) f -> p t f", p=128)
            for rep, fs in product(range(cc_reps), range(0, nt, fchunk)):
                fe = fs + fchunk
                attm = pf.tile([128, fchunk, 1], F32, tag="attm", bufs=1)
                nc.vector.tensor_scalar_max(out=attm[:],
                                            in0=o_all[:, fs:fe, 64:65],
                                            scalar1=1e-30)
                arec = pf.tile([128, fchunk, 1], F32, tag="arec", bufs=1)
                nc.vector.reciprocal(out=arec[:], in_=attm[:])
                o3 = pf.tile([128, fchunk, 64], F32, tag="o3", bufs=1)
                nc.vector.tensor_tensor(
                    out=o3[:], in0=o_all[:, fs:fe, 0:64],
                    in1=arec[:].broadcast_to([128, fchunk, 64]),
                    op=mybir.AluOpType.mult)
                nc.vector.tensor_tensor(out=o3[:], in0=o3[:],
                                        in1=xp_all[:, fs:fe, :],
                                        op=mybir.AluOpType.add)
                sq = pf.tile([128, fchunk, 64], F32, tag="sq", bufs=1)
                nc.vector.tensor_tensor(out=sq[:], in0=o3[:], in1=o3[:],
                                        op=mybir.AluOpType.mult)
                rs = pf.tile([128, fchunk, 1], F32, tag="rs", bufs=1)
                nc.vector.reduce_sum(out=rs[:], in_=sq[:],
                                     axis=mybir.AxisListType.X)
                rn = pf.tile([128, fchunk, 1], F32, tag="rn", bufs=1)
                nc.scalar.sqrt(out=rn[:], in_=rs[:])
                rnm = pf.tile([128, fchunk, 1], F32, tag="rnm", bufs=1)
                nc.vector.tensor_scalar_max(out=rnm[:], in0=rn[:], scalar1=1e-30)
                rrec = pf.tile([128, fchunk, 1], F32, tag="rrec", bufs=1)
                nc.vector.reciprocal(out=rrec[:], in_=rnm[:])
                nc.vector.tensor_tensor(
                    out=o3[:], in0=o3[:],
                    in1=rrec[:].broadcast_to([128, fchunk, 64]),
                    op=mybir.AluOpType.mult)
                nc.sync.dma_start(out=out_view[:, fs:fe, :], in_=o3[:])

    nc.compile()
    return nc


_NC_CACHE = {}


def kernel(**inputs) -> np.ndarray:
    """Full inputs in, full output out. Shards across 8 NeuronCores internally."""
    X = np.asarray(inputs["X"], dtype=np.float32)
    W = np.asarray(inputs["W"], dtype=np.float32)
    homo = np.asarray(inputs["homo"], dtype=np.float32)
    vertex = np.asarray(inputs["vertex"])
    edges = np.asarray(inputs["edges"])
    cfg = Cfg.from_inputs(vertex, edges)
    assert X.shape == (cfg.N, 64) and homo.shape == (cfg.E,)

    key = cfg
    if key not in _NC_CACHE:
        _NC_CACHE[key] = build_nc(cfg)
    nc = _NC_CACHE[key]

    in_maps = [prep_core_inputs(cfg, k, X, W, homo, vertex, edges)
               for k in range(cfg.n_cores)]
    res = bass_utils.run_bass_kernel_spmd(
        nc, in_maps, core_ids=list(range(cfg.n_cores)))
    out = np.concatenate(
        [res.results[k]["out"][:cfg.npc] for k in range(cfg.n_cores)], axis=0)
    return out.astype(np.float32)


# revision 31
# speedup vs baseline: 1.0935x; 1.0340x over previous
"""HyperGNN message-passing kernel v5 (nn_Conv_13778255086166) for 8 TRN2 cores.

Reference computation:
    Xp    = X @ W                                   [N, 64]
    Xe_s  = segment_sum(Xp[vertex], edges, E);  cnt = segment_sum(1, edges, E)
    Ze    = (homo / max(cnt,1)) * Xe_s              [E, 64]
    att_s = segment_sum(homo[edges], vertex, N)
    Xv    = segment_sum(Ze[edges], vertex, N) / att_s
    out   = row_l2_normalize(Xp + Xv)

Distribution: incidence lists sharded by vertex range (core k owns nodes
[k*12500, (k+1)*12500)); per-core edge partials AllReduced (bf16).

v5 vs v2 baseline (59ms -> ~1.7ms measured sustained per-exec):
  - variable per-tile slot caps (exact counts rounded to 128) instead of one
    global max cap: ~15% fewer gathered rows / masks / matmuls
  - one-hot masks built with pair-duplicated offsets so every DVE operand has
    a packed last dim -> 2x DVE mode (broadcast last-dim stride-0 disables it)
  - offsets for all tiles preloaded in one DMA (partition-major host layout)
  - pair-of-tiles processing: one idx DMA / one mask build / one psum->SBUF
    copy per pair; att column folded into the phase-2 psum (65-wide)
  - gathers ~768 idx each rotating the 4 SWDGE queues (tuned on HW)
  - staged SBUF accumulators with one strided DMA per phase; batched finalize
"""

from dataclasses import dataclass
from itertools import product

import numpy as np

import concourse.bacc as bacc
import concourse.mybir as mybir
import concourse.tile as tile
from concourse import bass_utils

F32 = mybir.dt.float32
BF16 = mybir.dt.bfloat16
I16 = mybir.dt.int16


@dataclass(frozen=True)
class Cfg:
    n_cores: int = 8
    N: int = 100000
    E: int = 25000
    caps1: tuple = ()   # per-edge-tile slot caps (multiples of 128)
    caps2: tuple = ()   # per-node-tile slot caps

    @staticmethod
    def from_inputs(vertex, edges, n_cores=8, N=100000, E=25000):
        """Exact per-tile slot caps (max over cores, rounded to 128)."""
        vertex = np.asarray(vertex).astype(np.int64)
        edges = np.asarray(edges).astype(np.int64)
        npc = N // n_cores
        etiles = ((E + 1 + 127) // 128 * 128) // 128
        ntiles = ((npc + 1 + 127) // 128 * 128) // 128
        m1 = np.zeros(etiles, np.int64)
        m2 = np.zeros(ntiles, np.int64)
        for k in range(n_cores):
            sel = (vertex >= k * npc) & (vertex < (k + 1) * npc)
            v_l, e_l = vertex[sel] - k * npc, edges[sel]
            m1 = np.maximum(m1, np.bincount(e_l >> 7, minlength=etiles))
            m2 = np.maximum(m2, np.bincount(v_l >> 7, minlength=ntiles))
        r = lambda x: int(max(128, (x + 127) // 128 * 128))
        return Cfg(n_cores=n_cores, N=N, E=E,
                   caps1=tuple(r(x) for x in m1),
                   caps2=tuple(r(x) for x in m2))

    @property
    def npc(self):
        assert self.N % self.n_cores == 0
        return self.N // self.n_cores

    @property
    def npcp(self):  # padded, with at least one spare zero row
        return (self.npc + 1 + 127) // 128 * 128

    @property
    def ntiles(self):
        return self.npcp // 128

    @property
    def ep(self):
        return (self.E + 1 + 127) // 128 * 128

    @property
    def etiles(self):
        return self.ep // 128


def _bf16():
    import ml_dtypes
    return ml_dtypes.bfloat16


def wrap_idx(idx: np.ndarray) -> np.ndarray:
    """int16 index layout for dma_gather: element j at [j%16, j//16],
    replicated across the 8 16-partition groups (one per Q7 cpu)."""
    s = idx.shape[0]
    assert s % 16 == 0
    w = np.ascontiguousarray(idx.astype(np.int16).reshape(-1, 16).T)
    return np.tile(w, (8, 1))


def prep_core_inputs(cfg: Cfg, k: int, X, W, homo, vertex, edges):
    """Host-side shard/sort/pad for core k (index/layout reorganization only)."""
    bf16 = _bf16()
    npc, npcp = cfg.npc, cfg.npcp
    vertex = np.asarray(vertex)
    edges = np.asarray(edges)
    sel = (vertex >= k * npc) & (vertex < (k + 1) * npc)
    v_l = (vertex[sel] - k * npc).astype(np.int64)
    e_l = edges[sel].astype(np.int64)

    def build(seg, other, tiles_n, caps, pad_gather):
        caps = np.asarray(caps)
        o = np.argsort(seg, kind="stable")
        s, g = seg[o], other[o]
        t_of = s >> 7
        counts = np.bincount(t_of, minlength=tiles_n)
        assert (counts <= caps).all(), (counts.max(), caps.max())
        starts = np.cumsum(counts) - counts
        capoff = np.cumsum(caps) - caps
        rank = np.arange(len(s)) - starts[t_of]
        dest = capoff[t_of] + rank
        S = int(caps.sum())
        gi = np.full(S, pad_gather, np.int64)
        off = np.zeros(S, np.float32)
        gi[dest] = g
        off[dest] = (s & 127).astype(np.float32)
        # flat idx blocks (pair-of-tiles contiguous, partition-major within
        # a pair), partition-major paired offsets
        wraps = [wrap_idx(gi[capoff[t]:capoff[t] + caps[t]])
                 for t in range(tiles_n)]
        idx_flat = np.concatenate(
            [np.hstack(wraps[q:q + 2]).ravel()
             for q in range(0, tiles_n, 2)])
        offp = np.concatenate(
            [off[capoff[t]:capoff[t] + caps[t]].reshape(caps[t] // 128, 128).T
             for t in range(tiles_n)], axis=1)          # [128, sum_c]
        off_pm = np.repeat(offp, 2, axis=1).astype(bf16)  # [128, sum_c*2]
        return idx_flat, np.ascontiguousarray(off_pm)

    # P1: segment by edge, gather by local vertex; pads gather zero row npc.
    g1, off1 = build(e_l, v_l, cfg.etiles, cfg.caps1, pad_gather=npc)
    # P2: segment by local vertex, gather by edge; pads gather zero row E.
    g2, off2 = build(v_l, e_l, cfg.ntiles, cfg.caps2, pad_gather=cfg.E)

    Xt = np.zeros((64, npcp), np.float32)
    Xt[:, :npc] = np.asarray(X)[k * npc:(k + 1) * npc].T

    homo_pad = np.zeros(cfg.ep, np.float32)
    homo_pad[:cfg.E] = np.asarray(homo)
    homo_t = np.ascontiguousarray(homo_pad.reshape(cfg.etiles, 128).T)

    # global per-edge incidence counts: pure index data -> host computes
    cnt = np.bincount(edges.astype(np.int64), minlength=cfg.E).astype(np.float32)
    cntr_pad = np.zeros(cfg.ep, np.float32)
    cntr_pad[:cfg.E] = 1.0 / np.maximum(cnt, 1.0)
    cntr_t = np.ascontiguousarray(cntr_pad.reshape(cfg.etiles, 128).T)

    iota = np.broadcast_to(np.arange(128, dtype=np.float32),
                           (128, 128)).astype(bf16).copy()

    return {
        "Xt": Xt,
        "W": np.asarray(W, dtype=np.float32),
        "homo_t": homo_t,
        "cntr_t": cntr_t,
        "iota": iota,
        "g1": g1,
        "off1": off1,
        "g2": g2,
        "off2": off2,
    }


def build_nc(cfg: Cfg, for_sim: bool = False, variant: str = "full",
             repeat: int = 1, gsplit: int = 6):
    """variant: full | nocc | p0 | p1 | p2 | p1n | p2n | nog"""
    no_g1 = variant in ("p1n", "nog")
    no_g2 = variant in ("p2n", "nog")
    variant = {"p1n": "p1", "p2n": "p2", "nog": "nocc"}.get(variant, variant)
    caps1, caps2 = cfg.caps1, cfg.caps2
    cs1 = [c // 128 for c in caps1]
    cs2 = [c // 128 for c in caps2]
    c1m, c2m = max(cs1), max(cs2)
    co1 = np.cumsum([0] + cs1)   # per-tile column offsets (128-slot units)
    co2 = np.cumsum([0] + cs2)
    sum1, sum2 = int(co1[-1]), int(co2[-1])
    nt, et = cfg.ntiles, cfg.etiles
    nc = bacc.Bacc("TRN2", target_bir_lowering=False, debug=False,
                   num_devices=1 if for_sim else cfg.n_cores,
                   num_swdge_queues=4, dynamic_dma_scratch_size=32768)

    xt_d = nc.dram_tensor("Xt", [64, cfg.npcp], F32, kind="ExternalInput")
    w_d = nc.dram_tensor("W", [64, 64], F32, kind="ExternalInput")
    homo_d = nc.dram_tensor("homo_t", [128, et], F32, kind="ExternalInput")
    cntr_d = nc.dram_tensor("cntr_t", [128, et], F32, kind="ExternalInput")
    iota_d = nc.dram_tensor("iota", [128, 128], BF16, kind="ExternalInput")
    g1_d = nc.dram_tensor("g1", [sum1 * 1024], I16, kind="ExternalInput")
    off1_d = nc.dram_tensor("off1", [128, sum1 * 2], BF16, kind="ExternalInput")
    g2_d = nc.dram_tensor("g2", [sum2 * 1024], I16, kind="ExternalInput")
    off2_d = nc.dram_tensor("off2", [128, sum2 * 2], BF16, kind="ExternalInput")
    out_d = nc.dram_tensor("out", [cfg.npcp, 64], F32, kind="ExternalOutput")

    xp_d = nc.dram_tensor("XpD", [cfg.npcp, 128], BF16, kind="Internal")
    eh = cfg.ep // 2   # edge-half boundary (tile-aligned)
    eacca_d = nc.dram_tensor("EaccAD", [eh, 64], BF16, kind="Internal")
    eaccb_d = nc.dram_tensor("EaccBD", [cfg.ep - eh, 64], BF16, kind="Internal")
    ereda_d = nc.dram_tensor("EredAD", [eh, 64], BF16, kind="Internal",
                             addr_space="Shared")
    eredb_d = nc.dram_tensor("EredBD", [cfg.ep - eh, 64], BF16, kind="Internal",
                             addr_space="Shared")
    zef_d = nc.dram_tensor("ZeFD", [cfg.ep, 128], BF16, kind="Internal")

    with tile.TileContext(nc) as tc:
        xtiles = next(d for d in range(min(7, nt), 0, -1) if nt % d == 0)
        xchunk = xtiles * 128  # phase-0 X streamed in nt/xtiles chunks
        zchunk = next(d for d in range(min(7, et), 0, -1) if et % d == 0)
        fchunk = next(d for d in range(min(14, nt), 0, -1) if nt % d == 0)
        with (
            tc.tile_pool(name="const", bufs=1) as pc,
            tc.tile_pool(name="xin", bufs=2) as px,
            tc.tile_pool(name="idx", bufs=8) as pidx,
            tc.tile_pool(name="gather", bufs=4) as pg,
            tc.tile_pool(name="onehot", bufs=3) as pm,
            tc.tile_pool(name="ze", bufs=2) as pz,
            tc.tile_pool(name="fin", bufs=2) as pf,
            tc.tile_pool(name="psum", bufs=4, space="PSUM") as pp,
        ):
            w_sb = pc.tile([64, 64], F32)
            nc.sync.dma_start(out=w_sb[:], in_=w_d[:])
            iota_sb = pc.tile([128, 64, 2], BF16)
            nc.sync.dma_start(
                out=iota_sb[:],
                in_=iota_d[:].rearrange("p (a b) -> p a b", b=2))
            homo_sb = pc.tile([128, et], F32)
            nc.sync.dma_start(out=homo_sb[:], in_=homo_d[:])
            cntr_sb = pc.tile([128, et], F32)
            nc.sync.dma_start(out=cntr_sb[:], in_=cntr_d[:])
            off1_sb = pc.tile([128, sum1, 2], BF16)
            nc.scalar.dma_start(
                out=off1_sb[:],
                in_=off1_d[:].rearrange("p (a b) -> p a b", b=2))
            off2_sb = pc.tile([128, sum2, 2], BF16)
            nc.scalar.dma_start(
                out=off2_sb[:],
                in_=off2_d[:].rearrange("p (a b) -> p a b", b=2))

            # persistent SBUF stages
            xp_all = pc.tile([128, nt, 64], F32)    # f32 Xp for phase-2 add
            acc_all = pc.tile([128, et, 64], BF16)  # phase-1 edge partials
            o_all = pc.tile([128, nt, 65], F32)     # phase-2 sums (col 64=att)

            # phase 0: Xp = X_local @ W -> xp_all (SBUF) and xp_d (DRAM, bf16)
            xp_view = xp_d.ap().rearrange("(t p) f -> p t f", p=128)
            for rep, cs in product(range(repeat), range(0, cfg.npcp, xchunk)):
                xc = px.tile([64, xchunk], F32, tag="xc")
                nc.sync.dma_start(out=xc[:], in_=xt_d[:, cs:cs + xchunk])
                xq = px.tile([128, xtiles, 64], BF16, tag="xq")
                for u in range(xtiles):
                    t = cs // 128 + u
                    ps = pp.tile([128, 65], F32, tag="ps", bufs=2)
                    nc.tensor.matmul(ps[:, 0:64], lhsT=xc[:, u * 128:(u + 1) * 128],
                                     rhs=w_sb[:], start=True, stop=True)
                    nc.vector.tensor_copy(out=xp_all[:, t, :], in_=ps[:, 0:64])
                    nc.scalar.copy(out=xq[:, u, :], in_=ps[:, 0:64])
                nc.sync.dma_start(
                    out=xp_view[:, cs // 128:cs // 128 + xtiles, 0:64], in_=xq[:])

            # phases 1/2: pair-of-tiles granularity. One idx DMA, one mask
            # is_equal, ~768-idx gathers (crossing tile boundaries), one
            # psum->SBUF copy per pair. Pipelined one pair ahead so the
            # in-order DVE/Q7 queues stay ahead of the PE matmul stream.
            LA = 1
            p1_reps = 0 if variant in ("p0", "p2") else repeat
            qctr = [0]

            def stage_pair(q, cs, co, pw, src_d, idx_d, off_sb, gtag, ctag,
                           no_g):
                t0 = 2 * q
                tiles = list(range(t0, min(t0 + 2, len(cs))))
                o = int(co[t0])
                C = sum(cs[t] for t in tiles)
                gi = pidx.tile([128, 8 * pw], I16, tag="i" + gtag)
                nc.sync.dma_start(
                    out=gi[:, 0:8 * C],
                    in_=idx_d[o * 1024:(o + C) * 1024]
                        .rearrange("(p c) -> p c", p=128))
                g = pg.tile([128, pw, 128], BF16, tag=gtag, bufs=2)
                if no_g:
                    nc.vector.memset(g[:, 0:C, :], 0.0)
                else:
                    nsp = max(1, (C + gsplit - 1) // gsplit)
                    for h in range(nsp):
                        a = (h * C) // nsp
                        b = ((h + 1) * C) // nsp
                        if a == b:
                            continue
                        nc.gpsimd.dma_gather(
                            g[:, a:b, :], src_d[:], gi[:, a * 8:b * 8],
                            (b - a) * 128, (b - a) * 128, 128,
                            single_packet=False,
                            queue_num=qctr[0] % 4)
                        qctr[0] += 1
                mt = pm.tile([128, pw, 64, 2], BF16, tag=ctag, bufs=2)
                nc.vector.tensor_tensor(
                    out=mt[:, 0:C, :, :],
                    in0=iota_sb[:].unsqueeze(1).broadcast_to([128, C, 64, 2]),
                    in1=off_sb[:, o:o + C, :].unsqueeze(2)
                        .broadcast_to([128, C, 64, 2]),
                    op=mybir.AluOpType.is_equal)
                return g, mt

            def run_phase(cs, co, pw, src_d, idx_d, off_sb, gtag, ctag, no_g,
                          fcols, sink, mid=None):
                ntl = len(cs)
                nq = (ntl + 1) // 2

                def stage(q):
                    return stage_pair(q, cs, co, pw, src_d, idx_d, off_sb,
                                      gtag, ctag, no_g)

                pend = {q: stage(q) for q in range(min(LA + 1, nq))}
                for q in range(nq):
                    if mid is not None and q == (nq + 1) // 2:
                        mid()
                    if q + LA + 1 < nq:
                        pend[q + LA + 1] = stage(q + LA + 1)
                    g, mt = pend.pop(q)
                    tiles = list(range(2 * q, min(2 * q + 2, ntl)))
                    ps = pp.tile([128, 2, 65], F32, tag="ps2t")
                    jj = 0
                    for i, t in enumerate(tiles):
                        for j in range(cs[t]):
                            nc.tensor.matmul(
                                ps[:, i, 0:fcols],
                                lhsT=mt[:, jj, :, :]
                                    .rearrange("p a b -> p (a b)"),
                                rhs=g[:, jj, 0:fcols],
                                start=(j == 0), stop=(j == cs[t] - 1))
                            jj += 1
                    sink(ps, tiles)

            def sink1(ps, tiles):
                nc.vector.tensor_copy(
                    out=acc_all[:, tiles[0]:tiles[0] + len(tiles), :],
                    in_=ps[:, 0:len(tiles), 0:64])

            cc_now = variant not in ("p0", "p1")
            eta = eh // 128
            eacca_view = eacca_d.ap().rearrange("(t p) f -> p t f", p=128)
            eaccb_view = eaccb_d.ap().rearrange("(t p) f -> p t f", p=128)

            def reduce_half(acc_view, acc_d, red_d, lo, hi):
                nc.sync.dma_start(out=acc_view[:],
                                  in_=acc_all[:, lo:hi, :])
                if not cc_now:
                    return
                if for_sim or variant in ("nocc", "p2"):
                    nc.sync.dma_start(out=red_d[:], in_=acc_d[:])
                else:
                    nc.gpsimd.collective_compute(
                        "AllReduce", mybir.AluOpType.add,
                        replica_groups=[list(range(cfg.n_cores))],
                        ins=[acc_d.ap()], outs=[red_d.ap()],
                    )

            def mid1():
                reduce_half(eacca_view, eacca_d, ereda_d, 0, eta)

            for rep in range(p1_reps):
                run_phase(cs1, co1, 2 * c1m, xp_d, g1_d, off1_sb, "g1", "mt1",
                          no_g1, 64, sink1, mid=mid1)
                reduce_half(eaccb_view, eaccb_d, eredb_d, eta, et)

            cc_reps = 0 if variant in ("p0", "p1") else repeat
            if variant in ("p0", "p1"):
                pass
            elif p1_reps == 0:
                # timing variants that skip phase 1 still need ered populated
                for rep in range(cc_reps):
                    mid1()
                    reduce_half(eaccb_view, eaccb_d, eredb_d, eta, et)

            # Ze build: zef rows = [Ye*homo | homo | junk], chunk-batched
            scale_sb = pc.tile([128, et], F32)
            nc.vector.tensor_tensor(out=scale_sb[:], in0=homo_sb[:],
                                    in1=cntr_sb[:], op=mybir.AluOpType.mult)
            era_view = ereda_d.ap().rearrange("(t p) f -> p t f", p=128)
            erb_view = eredb_d.ap().rearrange("(t p) f -> p t f", p=128)
            zf_view = zef_d.ap().rearrange("(t p) f -> p t f", p=128)
            for rep, zs in product(range(cc_reps), range(0, et, zchunk)):
                er = pz.tile([128, zchunk, 64], BF16, tag="er")
                if zs + zchunk <= eta:
                    erv = era_view[:, zs:zs + zchunk, :]
                else:
                    erv = erb_view[:, zs - eta:zs - eta + zchunk, :]
                nc.sync.dma_start(out=er[:], in_=erv)
                z = pz.tile([128, zchunk, 128], BF16, tag="z")
                nc.vector.tensor_tensor(
                    out=z[:, :, 0:64], in0=er[:],
                    in1=scale_sb[:, zs:zs + zchunk].unsqueeze(2)
                        .broadcast_to([128, zchunk, 64]),
                    op=mybir.AluOpType.mult)
                nc.vector.tensor_copy(
                    out=z[:, :, 64:65],
                    in_=homo_sb[:, zs:zs + zchunk].unsqueeze(2))
                nc.sync.dma_start(out=zf_view[:, zs:zs + zchunk, :], in_=z[:])

            # phase 2: node-tile accumulation (same pairing as phase 1)
            def sink2(ps, tiles):
                nc.vector.tensor_copy(
                    out=o_all[:, tiles[0]:tiles[0] + len(tiles), :],
                    in_=ps[:, 0:len(tiles), :])

            for rep in range(cc_reps):
                run_phase(cs2, co2, 2 * c2m, zef_d, g2_d, off2_sb, "g2", "mt2",
                          no_g2, 65, sink2)

            # batched finalize, fchunk node tiles at a time
            out_view = out_d.ap().rearrange("(t p) f -> p t f", p=128)
            for rep, fs in product(range(cc_reps), range(0, nt, fchunk)):
                fe = fs + fchunk
                attm = pf.tile([128, fchunk, 1], F32, tag="attm", bufs=1)
                nc.vector.tensor_scalar_max(out=attm[:],
                                            in0=o_all[:, fs:fe, 64:65],
                                            scalar1=1e-30)
                arec = pf.tile([128, fchunk, 1], F32, tag="arec", bufs=1)
                nc.vector.reciprocal(out=arec[:], in_=attm[:])
                o3 = pf.tile([128, fchunk, 64], F32, tag="o3", bufs=1)
                nc.vector.tensor_tensor(
                    out=o3[:], in0=o_all[:, fs:fe, 0:64],
                    in1=arec[:].broadcast_to([128, fchunk, 64]),
                    op=mybir.AluOpType.mult)
                nc.vector.tensor_tensor(out=o3[:], in0=o3[:],
                                        in1=xp_all[:, fs:fe, :],
                                        op=mybir.AluOpType.add)
                sq = pf.tile([128, fchunk, 64], F32, tag="sq", bufs=1)
                nc.vector.tensor_tensor(out=sq[:], in0=o3[:], in1=o3[:],
                                        op=mybir.AluOpType.mult)
                rs = pf.tile([128, fchunk, 1], F32, tag="rs", bufs=1)
                nc.vector.reduce_sum(out=rs[:], in_=sq[:],
                                     axis=mybir.AxisListType.X)
                rn = pf.tile([128, fchunk, 1], F32, tag="rn", bufs=1)
                nc.scalar.sqrt(out=rn[:], in_=rs[:])
                rnm = pf.tile([128, fchunk, 1], F32, tag="rnm", bufs=1)
                nc.vector.tensor_scalar_max(out=rnm[:], in0=rn[:], scalar1=1e-30)
                rrec = pf.tile([128, fchunk, 1], F32, tag="rrec", bufs=1)
                nc.vector.reciprocal(out=rrec[:], in_=rnm[:])
                nc.vector.tensor_tensor(
                    out=o3[:], in0=o3[:],
                    in1=rrec[:].broadcast_to([128, fchunk, 64]),
                    op=mybir.AluOpType.mult)
                nc.sync.dma_start(out=out_view[:, fs:fe, :], in_=o3[:])

    nc.compile()
    return nc


_NC_CACHE = {}


def kernel(**inputs) -> np.ndarray:
    """Full inputs in, full output out. Shards across 8 NeuronCores internally."""
    X = np.asarray(inputs["X"], dtype=np.float32)
    W = np.asarray(inputs["W"], dtype=np.float32)
    homo = np.asarray(inputs["homo"], dtype=np.float32)
    vertex = np.asarray(inputs["vertex"])
    edges = np.asarray(inputs["edges"])
    cfg = Cfg.from_inputs(vertex, edges)
    assert X.shape == (cfg.N, 64) and homo.shape == (cfg.E,)

    key = cfg
    if key not in _NC_CACHE:
        _NC_CACHE[key] = build_nc(cfg)
    nc = _NC_CACHE[key]

    in_maps = [prep_core_inputs(cfg, k, X, W, homo, vertex, edges)
               for k in range(cfg.n_cores)]
    res = bass_utils.run_bass_kernel_spmd(
        nc, in_maps, core_ids=list(range(cfg.n_cores)))
    out = np.concatenate(
        [res.results[k]["out"][:cfg.npc] for k in range(cfg.n_cores)], axis=0)
    return out.astype(np.float32)
